# revision 7
# baseline (speedup 1.0000x reference)
"""Trainium2 Bass kernel for DeChunking EMA (lower-triangular decay matmul).

Math: out[b,i,:] = sum_{j<=i} exp(S_i - S_j) * p_j * z[b,j,:],
with S = cumsum(log(clip(1-p))). Computed chunked-scan style (Mamba-SSD):

  - L split into C=32 chunks of Q=128.
  - Intra-chunk: out_intra = W_c^T.T @ z_c with
      W_c^T[j,i] = exp(S_i - S_j + log p_j) (masked i>=j),
    where the delta matrix S_i - S_j + log p_j is produced by a single K=3
    matmul of stacked [1, -S, logp] x [S, 1, 1] operands.
  - Inter-chunk: chunk states H_c = U_c^T @ z_c (U_c[j] = exp(Send_c - S_j
    + log p_j)), carried across chunks with one [32,32] decay matmul
    (carry = M2^T @ H), then applied per chunk as a rank-1 PSUM-accumulated
    matmul out += A_c (x) carry_c.

All exp inputs are <= 0 by construction, so nothing overflows.

Sharding (8 cores, no collectives): core = (batch b in {0,1}) x (one of 4
D-blocks of 192). Each core reads z[b, :, blk] and pt[b] only.
"""

import os
import numpy as np

B, L, D = 2, 4096, 768
Q = 128
C = L // Q           # 32 chunks
ND = 4               # D blocks per batch
DBLK = D // ND       # 192
GRP = 4              # chunks per exp/mask group
NG = C // GRP        # 8 groups
NEG = -3.0e38
N_CORES = 8

_CTX = {}
LAST_EXEC_NS = None


def _build_program():
    import concourse.bacc as bacc
    import concourse.mybir as mybir
    from concourse import tile

    f32 = mybir.dt.float32
    nc = bacc.Bacc("TRN2", target_bir_lowering=False, debug=False,
                   num_devices=N_CORES)

    z_s = nc.dram_tensor("z_s", [C, Q, DBLK], f32, kind="ExternalInput")
    stackL = nc.dram_tensor("stackL", [3, L], f32, kind="ExternalInput")
    stackR = nc.dram_tensor("stackR", [3, L], f32, kind="ExternalInput")
    uexp = nc.dram_tensor("uexp", [Q, C], f32, kind="ExternalInput")
    aexp = nc.dram_tensor("aexp", [C, Q], f32, kind="ExternalInput")
    d2exp = nc.dram_tensor("d2exp", [C, C], f32, kind="ExternalInput")
    maskb = nc.dram_tensor("maskb", [Q, GRP * Q], f32, kind="ExternalInput")
    ident = nc.dram_tensor("ident", [Q, Q], f32, kind="ExternalInput")
    out_s = nc.dram_tensor("out_s", [C, Q, DBLK], f32, kind="ExternalOutput")

    Exp = mybir.ActivationFunctionType.Exp
    HD = DBLK // 2  # 96: half of a D block, so M <= 128 in the H matvecs

    with tile.TileContext(nc) as tc:
        with (
            tc.tile_pool(name="zp", bufs=C) as zp,
            tc.tile_pool(name="wp", bufs=NG) as wp,
            tc.tile_pool(name="sp", bufs=1) as sp,
            tc.tile_pool(name="op", bufs=4) as op,
            tc.tile_pool(name="dps", bufs=2, space="PSUM") as dps,
            tc.tile_pool(name="ops", bufs=2, space="PSUM") as ops,
            tc.tile_pool(name="hps", bufs=1, space="PSUM") as hps,
        ):
            # small operand loads
            sL = sp.tile([3, L], f32, tag="sL")
            nc.sync.dma_start(sL[:], stackL[:])
            sR = sp.tile([3, L], f32, tag="sR")
            nc.sync.dma_start(sR[:], stackR[:])
            ue = sp.tile([Q, C], f32, tag="ue")
            nc.sync.dma_start(ue[:], uexp[:])
            ae = sp.tile([C, Q], f32, tag="ae")
            nc.sync.dma_start(ae[:], aexp[:])
            d2 = sp.tile([C, C], f32, tag="d2")
            nc.sync.dma_start(d2[:], d2exp[:])
            mb = sp.tile([Q, GRP * Q], f32, tag="mb")
            nc.sync.dma_start(mb[:], maskb[:])
            idn = sp.tile([Q, Q], f32, tag="idn")
            nc.sync.dma_start(idn[:], ident[:])

            U = sp.tile([Q, C], f32, tag="U")
            nc.scalar.activation(U[:], ue[:], Exp)
            A = sp.tile([C, Q], f32, tag="A")
            nc.scalar.activation(A[:], ae[:], Exp)
            M2 = sp.tile([C, C], f32, tag="M2")
            nc.scalar.activation(M2[:], d2[:], Exp)

            # A rows flattened to one partition so per-chunk rank-1 matmuls
            # can read [1, Q] slices at base partition 0
            aflat = sp.tile([1, C * Q], f32, tag="aflat")
            nc.sync.dma_start(aflat[:], A[:])

            # z chunk loads + state contributions, computed transposed:
            # H_T[d, c] = z_c[:, d] . U[:, c]  (two 96-row halves)
            zc = []
            ht_ps = hps.tile([HD, 2 * C], f32, tag="ht")
            for c in range(C):
                t = zp.tile([Q, DBLK], f32, tag="z")
                nc.sync.dma_start(t[:], z_s[c])
                zc.append(t)
                nc.tensor.matmul(
                    ht_ps[:, c : c + 1], t[:, 0:HD], U[:, c : c + 1]
                )
                nc.tensor.matmul(
                    ht_ps[:, C + c : C + c + 1], t[:, HD:DBLK], U[:, c : c + 1]
                )

            ht_sb = sp.tile([HD, 2 * C], f32, tag="ht_sb")
            nc.vector.tensor_copy(ht_sb[:], ht_ps[:])
            # transpose the two halves back to chunk-major H [C, DBLK]
            h_tr = hps.tile([C, DBLK], f32, tag="h_tr")
            nc.tensor.transpose(h_tr[:, 0:HD], ht_sb[:, 0:C], idn[:HD, :HD])
            nc.tensor.transpose(h_tr[:, HD:DBLK], ht_sb[:, C : 2 * C], idn[:HD, :HD])
            H = sp.tile([C, DBLK], f32, tag="H")
            nc.vector.tensor_copy(H[:], h_tr[:])

            c_ps = hps.tile([C, DBLK], f32, tag="cps")
            nc.tensor.matmul(c_ps[:], M2[:], H[:])
            carry = sp.tile([C, DBLK], f32, tag="carry")
            nc.vector.tensor_copy(carry[:], c_ps[:])
            # flatten carry rows to one partition for the rank-1 matmuls
            cflat = sp.tile([1, C * DBLK], f32, tag="cflat")
            nc.sync.dma_start(cflat[:], carry[:])

            # W^T blocks: delta via K=3 matmul, mask, exp (grouped by GRP)
            wT = []
            for g in range(NG):
                dp = dps.tile([Q, GRP * Q], f32, tag="dp")
                for k in range(GRP):
                    c = g * GRP + k
                    nc.tensor.matmul(
                        dp[:, k * Q : (k + 1) * Q],
                        sL[:, c * Q : (c + 1) * Q],
                        sR[:, c * Q : (c + 1) * Q],
                    )
                nc.vector.tensor_add(dp[:], dp[:], mb[:])
                w4 = wp.tile([Q, GRP * Q], f32, tag="w4")
                nc.scalar.activation(w4[:], dp[:], Exp)
                wT.append(w4)

            # outputs: out_c = W_c^T.T @ z_c + A_c (x) carry_c
            for c in range(C):
                g, k = divmod(c, GRP)
                o_ps = ops.tile([Q, DBLK], f32, tag="o")
                nc.tensor.matmul(
                    o_ps[:], wT[g][:, k * Q : (k + 1) * Q], zc[c][:],
                    start=True, stop=False,
                )
                nc.tensor.matmul(
                    o_ps[:],
                    aflat[:, c * Q : (c + 1) * Q],
                    cflat[:, c * DBLK : (c + 1) * DBLK],
                    start=False, stop=True,
                )
                o_sb = op.tile([Q, DBLK], f32, tag="osb")
                if c % 2 == 0:
                    nc.scalar.copy(o_sb[:], o_ps[:])
                else:
                    nc.vector.tensor_copy(o_sb[:], o_ps[:])
                nc.sync.dma_start(out_s[c], o_sb[:])

    nc.compile()
    return nc


def _host_prep(pt_b):
    """Per-batch host-side prep of the small scan operands. pt_b: [L] f32."""
    pt_b = pt_b.astype(np.float64)
    decay = np.clip(1.0 - pt_b, 1e-12, None)
    S = np.cumsum(np.log(decay))
    logp = np.log(np.maximum(pt_b, 1e-38))
    Send = S[Q - 1 :: Q]
    Sendprev = np.concatenate([[0.0], Send[:-1]])

    stackL = np.stack([np.ones(L), -S, logp]).astype(np.float32)
    stackR = np.stack([S, np.ones(L), np.ones(L)]).astype(np.float32)

    Smat = S.reshape(C, Q)
    logpm = logp.reshape(C, Q)
    uexp = (Send[:, None] - Smat + logpm).T.astype(np.float32)
    aexp = (Smat - Sendprev[:, None]).astype(np.float32)
    m_i = np.arange(C)[:, None]
    c_i = np.arange(C)[None, :]
    d2exp = np.where(m_i < c_i, Sendprev[None, :] - Send[:, None], NEG)
    d2exp = d2exp.astype(np.float32)
    return stackL, stackR, uexp, aexp, d2exp


_MASKB = None


def _get_maskb():
    global _MASKB
    if _MASKB is None:
        j = np.arange(Q)[:, None]
        i = np.arange(Q)[None, :]
        one = np.where(i >= j, 0.0, NEG).astype(np.float32)
        _MASKB = np.tile(one, (1, GRP))
    return _MASKB


def _install_ntff_shim():
    """Enable NTFF profiling under axon: shim the missing antenv.axon_hooks
    module and register the ctypes hook from trn_boot; skip the fileshare
    artifact upload (no bucket in this container)."""
    import sys
    import types
    import antenv

    if "antenv.axon_hooks" not in sys.modules:
        mod = types.ModuleType("antenv.axon_hooks")
        hook_box = [None]
        mod.set_axon_ntff_profile_hook = lambda h: hook_box.__setitem__(0, h)
        mod.get_axon_ntff_profile_hook = lambda: hook_box[0]
        mod._hook_box = hook_box
        sys.modules["antenv.axon_hooks"] = mod
        antenv.axon_hooks = mod
    mod = sys.modules["antenv.axon_hooks"]
    if mod.get_axon_ntff_profile_hook() is None:
        from trn_agent_boot.trn_boot import _ntff_profile_via_ctypes

        mod.set_axon_ntff_profile_hook(
            _ntff_profile_via_ctypes("/opt/axon/libaxon_pjrt.so")
        )
    import concourse.bass_utils as bu

    bu.upload_artifacts = lambda tmpdir: f"local://{tmpdir}"


def kernel(z, pt):
    global LAST_EXEC_NS
    from concourse.bass_utils import run_bass_kernel_spmd

    z = np.asarray(z, dtype=np.float32)
    pt = np.asarray(pt, dtype=np.float32)

    if "nc" not in _CTX:
        _CTX["nc"] = _build_program()
    nc = _CTX["nc"]

    maskb = _get_maskb()
    preps = [_host_prep(pt[b]) for b in range(B)]
    in_maps = []
    for core in range(N_CORES):
        b, dblk = divmod(core, ND)
        stackL, stackR, uexp, aexp, d2exp = preps[b]
        z_slab = np.ascontiguousarray(
            z[b, :, dblk * DBLK : (dblk + 1) * DBLK]
        ).reshape(C, Q, DBLK)
        in_maps.append({
            "z_s": z_slab,
            "stackL": stackL,
            "stackR": stackR,
            "uexp": uexp,
            "aexp": aexp,
            "d2exp": d2exp,
            "maskb": maskb,
            "ident": np.eye(Q, dtype=np.float32),
        })

    trace = bool(int(os.environ.get("BASS_KERNEL_TRACE", "0")))
    if trace:
        try:
            _install_ntff_shim()
        except Exception:
            trace = False
    tmpdir = os.environ.get("BASS_KERNEL_TRACE_DIR") or None
    res = run_bass_kernel_spmd(
        nc, in_maps, list(range(N_CORES)), trace=trace, tmpdir=tmpdir
    )
    LAST_EXEC_NS = res.exec_time_ns

    out = np.empty((B, L, D), np.float32)
    for core in range(N_CORES):
        b, dblk = divmod(core, ND)
        out[b, :, dblk * DBLK : (dblk + 1) * DBLK] = (
            res.results[core]["out_s"].reshape(L, DBLK)
        )
    return out


# revision 14
# speedup vs baseline: 1.7646x; 1.7646x over previous
"""Trainium2 Bass kernel for DeChunking EMA (lower-triangular decay matmul).

Math: out[b,i,:] = sum_{j<=i} exp(S_i - S_j) * p_j * z[b,j,:],
with S = cumsum(log(clip(1-p))). Computed chunked-scan style (Mamba-SSD):

  - L split into C=32 chunks of Q=128.
  - Intra-chunk: out_intra = W_c^T.T @ z_c with
      W_c^T[j,i] = exp(S_i - S_j + log p_j) (masked to i>=j).
    The delta matrix is produced on PE by a block-diagonal stacked matmul
    (4 chunks per [128,512] PSUM group): delta = 1*S_i + (-S_j)*1 +
    logp_j*1, a K=3 product per chunk -> K=12 block-diagonal per group.
  - Inter-chunk: chunk states H_c = U_c^T @ z_c (computed transposed, z as
    weights, two 96-row halves), PE-transposed back, carried across chunks
    with one [32,32] decay matmul (carry = M2^T @ H), then applied per
    chunk as a rank-1 PSUM-accumulated matmul out += A_c (x) carry_c.

All exp inputs are <= 0 by construction, so nothing overflows. The decay
weights / z / state operands run in bf16 on the PE (fp32 PSUM accumulate);
the delta stack runs in fp32 (S spans thousands; bf16 would destroy it).

Sharding (8 cores, no collectives): core = (batch b in {0,1}) x (one of 4
D-blocks of 192). Each core reads z[b, :, blk] and pt[b] only.
"""

import os
import numpy as np
import ml_dtypes

B, L, D = 2, 4096, 768
Q = 128
C = L // Q           # 32 chunks
ND = 4               # D blocks per batch
DBLK = D // ND       # 192
GRP = 4              # chunks per delta/exp group
NG = C // GRP        # 8 groups
NEG = -3.0e38
N_CORES = 8

_CTX = {}
LAST_EXEC_NS = None


def _build_program():
    import concourse.bacc as bacc
    import concourse.mybir as mybir
    from concourse import tile

    f32 = mybir.dt.float32
    bf16 = mybir.dt.bfloat16
    nc = bacc.Bacc("TRN2", target_bir_lowering=False, debug=False,
                   num_devices=N_CORES)

    f32r = mybir.dt.float32r
    z_s = nc.dram_tensor("z_s", [C, Q, DBLK], bf16, kind="ExternalInput")
    stackL = nc.dram_tensor("stackL", [3 * GRP, NG * Q], f32r, kind="ExternalInput")
    stackR = nc.dram_tensor("stackR", [3 * GRP, NG * GRP * Q], f32r,
                            kind="ExternalInput")
    uexp = nc.dram_tensor("uexp", [Q, C], f32, kind="ExternalInput")
    aexp = nc.dram_tensor("aexp", [C, Q], f32, kind="ExternalInput")
    d2exp = nc.dram_tensor("d2exp", [C, C], f32, kind="ExternalInput")
    maskb = nc.dram_tensor("maskb", [Q, GRP * Q], f32, kind="ExternalInput")
    ident = nc.dram_tensor("ident", [Q, Q], bf16, kind="ExternalInput")
    out_s = nc.dram_tensor("out_s", [C, Q, DBLK], f32, kind="ExternalOutput")

    Exp = mybir.ActivationFunctionType.Exp
    HD = DBLK // 2  # 96: half of a D block, so M <= 128 in the H matvecs

    with tile.TileContext(nc) as tc:
        with (
            tc.tile_pool(name="zp", bufs=C) as zp,
            tc.tile_pool(name="wp", bufs=NG) as wp,
            tc.tile_pool(name="sp", bufs=1) as sp,
            tc.tile_pool(name="op", bufs=4) as op,
            tc.tile_pool(name="dps", bufs=2, space="PSUM") as dps,
            tc.tile_pool(name="ops", bufs=2, space="PSUM") as ops,
            tc.tile_pool(name="hps", bufs=1, space="PSUM") as hps,
        ):
            # small operand loads
            sL = sp.tile([3 * GRP, NG * Q], f32r, tag="sL")
            nc.sync.dma_start(sL[:], stackL[:])
            sR = sp.tile([3 * GRP, NG * GRP * Q], f32r, tag="sR")
            nc.sync.dma_start(sR[:], stackR[:])
            ue = sp.tile([Q, C], f32, tag="ue")
            nc.sync.dma_start(ue[:], uexp[:])
            ae = sp.tile([C, Q], f32, tag="ae")
            nc.sync.dma_start(ae[:], aexp[:])
            d2 = sp.tile([C, C], f32, tag="d2")
            nc.sync.dma_start(d2[:], d2exp[:])
            mb = sp.tile([Q, GRP * Q], f32, tag="mb")
            nc.sync.dma_start(mb[:], maskb[:])
            idn = sp.tile([Q, Q], bf16, tag="idn")
            nc.sync.dma_start(idn[:], ident[:])

            U = sp.tile([Q, C], bf16, tag="U")
            nc.scalar.activation(U[:], ue[:], Exp)
            A = sp.tile([C, Q], bf16, tag="A")
            nc.scalar.activation(A[:], ae[:], Exp)
            M2 = sp.tile([C, C], bf16, tag="M2")
            nc.scalar.activation(M2[:], d2[:], Exp)

            # A rows flattened to one partition so per-chunk rank-1 matmuls
            # can read [1, Q] slices at base partition 0
            aflat = sp.tile([1, C * Q], bf16, tag="aflat")
            nc.sync.dma_start(aflat[:], A[:])

            # z chunk loads + state contributions, computed transposed:
            # H_T[d, c] = z_c[:, d] . U[:, c]  (two 96-row halves)
            zc = []
            ht_ps = hps.tile([HD, 2 * C], f32, tag="ht")
            for c in range(C):
                t = zp.tile([Q, DBLK], bf16, tag="z")
                nc.sync.dma_start(t[:], z_s[c])
                zc.append(t)
                nc.tensor.matmul(
                    ht_ps[:, c : c + 1], t[:, 0:HD], U[:, c : c + 1]
                )
                nc.tensor.matmul(
                    ht_ps[:, C + c : C + c + 1], t[:, HD:DBLK], U[:, c : c + 1]
                )

            ht_sb = sp.tile([HD, 2 * C], bf16, tag="ht_sb")
            nc.vector.tensor_copy(ht_sb[:], ht_ps[:])
            # transpose the two halves back to chunk-major H [C, DBLK]
            h_tr = hps.tile([C, DBLK], bf16, tag="h_tr")
            nc.tensor.transpose(h_tr[:, 0:HD], ht_sb[:, 0:C], idn[:HD, :HD])
            nc.tensor.transpose(h_tr[:, HD:DBLK], ht_sb[:, C : 2 * C], idn[:HD, :HD])
            H = sp.tile([C, DBLK], bf16, tag="H")
            nc.vector.tensor_copy(H[:], h_tr[:])

            c_ps = hps.tile([C, DBLK], f32, tag="cps")
            nc.tensor.matmul(c_ps[:], M2[:], H[:])
            carry = sp.tile([C, DBLK], bf16, tag="carry")
            nc.vector.tensor_copy(carry[:], c_ps[:])
            # flatten carry rows to one partition for the rank-1 matmuls
            cflat = sp.tile([1, C * DBLK], bf16, tag="cflat")
            nc.sync.dma_start(cflat[:], carry[:])

            # W^T blocks: block-diagonal K=12 delta matmul per group of 4
            # chunks (float32r, N=512 so it runs at 1 cycle/row), mask, exp
            wT = []
            for g in range(NG):
                dp = dps.tile([Q, GRP * Q], f32, tag="dp")
                nc.tensor.matmul(
                    dp[:],
                    sL[:, g * Q : (g + 1) * Q],
                    sR[:, g * GRP * Q : (g + 1) * GRP * Q],
                )
                nc.vector.tensor_add(dp[:], dp[:], mb[:])
                w4 = wp.tile([Q, GRP * Q], bf16, tag="w4")
                nc.scalar.activation(w4[:], dp[:], Exp)
                wT.append(w4)

            # outputs: out_c = W_c^T.T @ z_c + A_c (x) carry_c
            for c in range(C):
                g, k = divmod(c, GRP)
                o_ps = ops.tile([Q, DBLK], f32, tag="o")
                nc.tensor.matmul(
                    o_ps[:], wT[g][:, k * Q : (k + 1) * Q], zc[c][:],
                    start=True, stop=False,
                )
                nc.tensor.matmul(
                    o_ps[:],
                    aflat[:, c * Q : (c + 1) * Q],
                    cflat[:, c * DBLK : (c + 1) * DBLK],
                    start=False, stop=True,
                )
                o_sb = op.tile([Q, DBLK], f32, tag="osb")
                if c % 2 == 0:
                    nc.scalar.copy(o_sb[:], o_ps[:])
                else:
                    nc.vector.tensor_copy(o_sb[:], o_ps[:])
                nc.sync.dma_start(out_s[c], o_sb[:])

    nc.compile()
    return nc


def _host_prep(pt_b):
    """Per-batch host-side prep of the small scan operands. pt_b: [L] f32."""
    pt_b = pt_b.astype(np.float64)
    decay = np.clip(1.0 - pt_b, 1e-12, None)
    S = np.cumsum(np.log(decay))
    logp = np.log(np.maximum(pt_b, 1e-38))
    Send = S[Q - 1 :: Q]
    Sendprev = np.concatenate([[0.0], Send[:-1]])

    # Block-diagonal stacked operands for the grouped delta matmul.
    # Group g covers chunks c = g*GRP + k'.  For output [j, k'*Q + i]:
    #   delta = 1 * S_i + (-S_j) * 1 + logp_j * 1   (rows 3k' .. 3k'+2)
    # lhsT rows depend on j (the chunk of column-block k'), rhs rows gate
    # the i blocks.
    stackL = np.zeros((3 * GRP, NG * Q), np.float32)
    stackR = np.zeros((3 * GRP, NG * GRP * Q), np.float32)
    Sm = S.reshape(C, Q)
    logpm = logp.reshape(C, Q)
    # Re-center S within each chunk: only within-chunk differences matter for
    # the intra-chunk delta, and small magnitudes survive the PE's reduced
    # fp32r mantissa. Also pre-round operands to bf16-hi+lo representable
    # values so the fp32r decomposition is exact.
    Sc = Sm - Sm[:, :1]

    def r16(x):
        h = x.astype(ml_dtypes.bfloat16).astype(np.float64)
        l = (x - h).astype(ml_dtypes.bfloat16).astype(np.float64)
        return h + l

    Sc = r16(Sc)
    logpr = r16(logpm)
    for g in range(NG):
        for k in range(GRP):
            c = g * GRP + k
            lcol = slice(g * Q, (g + 1) * Q)
            stackL[3 * k + 0, lcol] = 1.0
            stackL[3 * k + 1, lcol] = -Sc[c]
            stackL[3 * k + 2, lcol] = logpr[c]
            rcol = slice(g * GRP * Q + k * Q, g * GRP * Q + (k + 1) * Q)
            stackR[3 * k + 0, rcol] = Sc[c]
            stackR[3 * k + 1, rcol] = 1.0
            stackR[3 * k + 2, rcol] = 1.0

    uexp = (Send[:, None] - Sm + logpm).T.astype(np.float32)
    aexp = (Sm - Sendprev[:, None]).astype(np.float32)
    m_i = np.arange(C)[:, None]
    c_i = np.arange(C)[None, :]
    d2exp = np.where(m_i < c_i, Sendprev[None, :] - Send[:, None], NEG)
    d2exp = d2exp.astype(np.float32)
    return stackL, stackR, uexp, aexp, d2exp


_MASKB = None


def _get_maskb():
    global _MASKB
    if _MASKB is None:
        j = np.arange(Q)[:, None]
        i = np.arange(Q)[None, :]
        one = np.where(i >= j, 0.0, NEG).astype(np.float32)
        _MASKB = np.tile(one, (1, GRP))
    return _MASKB


def _install_ntff_shim():
    """Enable NTFF profiling under axon: shim the missing antenv.axon_hooks
    module and register the ctypes hook from trn_boot; skip the fileshare
    artifact upload (no bucket in this container)."""
    import sys
    import types
    import antenv

    if "antenv.axon_hooks" not in sys.modules:
        mod = types.ModuleType("antenv.axon_hooks")
        hook_box = [None]
        mod.set_axon_ntff_profile_hook = lambda h: hook_box.__setitem__(0, h)
        mod.get_axon_ntff_profile_hook = lambda: hook_box[0]
        mod._hook_box = hook_box
        sys.modules["antenv.axon_hooks"] = mod
        antenv.axon_hooks = mod
    mod = sys.modules["antenv.axon_hooks"]
    if mod.get_axon_ntff_profile_hook() is None:
        from trn_agent_boot.trn_boot import _ntff_profile_via_ctypes

        mod.set_axon_ntff_profile_hook(
            _ntff_profile_via_ctypes("/opt/axon/libaxon_pjrt.so")
        )
    import concourse.bass_utils as bu

    bu.upload_artifacts = lambda tmpdir: f"local://{tmpdir}"


def kernel(z, pt):
    global LAST_EXEC_NS
    from concourse.bass_utils import run_bass_kernel_spmd

    z = np.asarray(z, dtype=np.float32)
    pt = np.asarray(pt, dtype=np.float32)

    if "nc" not in _CTX:
        _CTX["nc"] = _build_program()
    nc = _CTX["nc"]

    maskb = _get_maskb()
    preps = [_host_prep(pt[b]) for b in range(B)]
    in_maps = []
    for core in range(N_CORES):
        b, dblk = divmod(core, ND)
        stackL, stackR, uexp, aexp, d2exp = preps[b]
        z_slab = np.ascontiguousarray(
            z[b, :, dblk * DBLK : (dblk + 1) * DBLK]
        ).reshape(C, Q, DBLK).astype(ml_dtypes.bfloat16)
        in_maps.append({
            "z_s": z_slab,
            "stackL": stackL,
            "stackR": stackR,
            "uexp": uexp,
            "aexp": aexp,
            "d2exp": d2exp,
            "maskb": maskb,
            "ident": np.eye(Q, dtype=ml_dtypes.bfloat16),
        })

    trace = bool(int(os.environ.get("BASS_KERNEL_TRACE", "0")))
    if trace:
        try:
            _install_ntff_shim()
        except Exception:
            trace = False
    tmpdir = os.environ.get("BASS_KERNEL_TRACE_DIR") or None
    res = run_bass_kernel_spmd(
        nc, in_maps, list(range(N_CORES)), trace=trace, tmpdir=tmpdir
    )
    LAST_EXEC_NS = res.exec_time_ns

    out = np.empty((B, L, D), np.float32)
    for core in range(N_CORES):
        b, dblk = divmod(core, ND)
        out[b, :, dblk * DBLK : (dblk + 1) * DBLK] = (
            res.results[core]["out_s"].reshape(L, DBLK)
        )
    return out


# revision 15
# speedup vs baseline: 2.2380x; 1.2683x over previous
"""Trainium2 Bass kernel for DeChunking EMA (lower-triangular decay matmul).

Math: out[b,i,:] = sum_{j<=i} exp(S_i - S_j) * p_j * z[b,j,:],
with S = cumsum(log(clip(1-p))). Computed chunked-scan style (Mamba-SSD):

  - L split into C=32 chunks of Q=128.
  - Intra-chunk: out_intra = W_c^T.T @ z_c with
      W_c^T[j,i] = exp(S'_i - S'_j + log p_j) (masked to i>=j),
    where S' is S re-centered per chunk (only within-chunk differences
    matter, and small magnitudes survive the PE's fp32r mantissa split).
    The delta matrix is produced on PE by a block-diagonal stacked fp32r
    matmul: delta = 1*S'_i + (-S'_j)*1 + logp_j*1, K=3 per chunk -> K=12
    block-diagonal over a group of 4 chunks ([128,512] PSUM, 1 cycle/row).
  - Inter-chunk: chunk states H_c = U_c^T @ z_c accumulate into one
    [32,192] PSUM tile via a block-diagonal U (zero except column c of
    each [128,32] slab), then one [32,32] decay matmul forms all carry-in
    rows (carry = M2^T @ H), applied per chunk as a rank-1 PSUM-accumulated
    matmul out += A_c (x) carry_c.

All exp inputs are <= 0 by construction, so nothing overflows. The decay
weights / z / state operands run in bf16 on the PE (fp32 PSUM accumulate);
the delta stack runs in fp32r (S' re-centered + pre-rounded to bf16 hi+lo).

DRAM layouts are position-major ([Q, C*DBLK]) so every DMA moves >=3 KiB
contiguous per partition.

Sharding (8 cores, no collectives): core = (batch b in {0,1}) x (one of 4
D-blocks of 192). Each core reads z[b, :, blk] and pt[b] only.
"""

import os
import numpy as np
import ml_dtypes

B, L, D = 2, 4096, 768
Q = 128
C = L // Q           # 32 chunks
ND = 4               # D blocks per batch
DBLK = D // ND       # 192
GRP = 4              # chunks per delta/exp group
NG = C // GRP        # 8 groups
NEG = -3.0e38
N_CORES = 8
NZDMA = 4            # z-load / out-store DMA splits

_CTX = {}
LAST_EXEC_NS = None


def _build_program():
    import concourse.bacc as bacc
    import concourse.mybir as mybir
    from concourse import tile

    f32 = mybir.dt.float32
    f32r = mybir.dt.float32r
    bf16 = mybir.dt.bfloat16
    nc = bacc.Bacc("TRN2", target_bir_lowering=False, debug=False,
                   num_devices=N_CORES)

    FD = C * DBLK  # 6144 free elems in the big position-major tiles
    z_s = nc.dram_tensor("z_s", [Q, FD], bf16, kind="ExternalInput")
    stackL = nc.dram_tensor("stackL", [3 * GRP, NG * Q], f32r, kind="ExternalInput")
    stackR = nc.dram_tensor("stackR", [3 * GRP, NG * GRP * Q], f32r,
                            kind="ExternalInput")
    uexpblk = nc.dram_tensor("uexpblk", [Q, C * C], f32, kind="ExternalInput")
    aexp = nc.dram_tensor("aexp", [C, Q], f32, kind="ExternalInput")
    d2exp = nc.dram_tensor("d2exp", [C, C], f32, kind="ExternalInput")
    maskb = nc.dram_tensor("maskb", [Q, GRP * Q], f32, kind="ExternalInput")
    out_s = nc.dram_tensor("out_s", [Q, FD], f32, kind="ExternalOutput")

    Exp = mybir.ActivationFunctionType.Exp

    with tile.TileContext(nc) as tc:
        with (
            tc.tile_pool(name="zp", bufs=1) as zp,
            tc.tile_pool(name="wp", bufs=NG) as wp,
            tc.tile_pool(name="sp", bufs=1) as sp,
            tc.tile_pool(name="dps", bufs=2, space="PSUM") as dps,
            tc.tile_pool(name="ops", bufs=3, space="PSUM") as ops,
            tc.tile_pool(name="hps", bufs=1, space="PSUM") as hps,
        ):
            # small operand loads
            sL = sp.tile([3 * GRP, NG * Q], f32r, tag="sL")
            nc.sync.dma_start(sL[:], stackL[:])
            sR = sp.tile([3 * GRP, NG * GRP * Q], f32r, tag="sR")
            nc.sync.dma_start(sR[:], stackR[:])
            ub = sp.tile([Q, C * C], f32, tag="ub")
            nc.sync.dma_start(ub[:], uexpblk[:])
            ae = sp.tile([C, Q], f32, tag="ae")
            nc.sync.dma_start(ae[:], aexp[:])
            d2 = sp.tile([C, C], f32, tag="d2")
            nc.sync.dma_start(d2[:], d2exp[:])
            mb = sp.tile([Q, GRP * Q], f32, tag="mb")
            nc.sync.dma_start(mb[:], maskb[:])

            Ublk = sp.tile([Q, C * C], bf16, tag="Ublk")
            nc.scalar.activation(Ublk[:], ub[:], Exp)
            A = sp.tile([C, Q], bf16, tag="A")
            nc.scalar.activation(A[:], ae[:], Exp)
            M2 = sp.tile([C, C], bf16, tag="M2")
            nc.scalar.activation(M2[:], d2[:], Exp)

            # A rows flattened to one partition so per-chunk rank-1 matmuls
            # can read [1, Q] slices at base partition 0
            aflat = sp.tile([1, C * Q], bf16, tag="aflat")
            nc.sync.dma_start(aflat[:], A[:])

            # z: one position-major SBUF tile, loaded in NZDMA slabs
            zall = zp.tile([Q, FD], bf16, tag="z")
            zsl = FD // NZDMA
            for s in range(NZDMA):
                nc.sync.dma_start(
                    zall[:, s * zsl : (s + 1) * zsl],
                    z_s[:, s * zsl : (s + 1) * zsl],
                )

            # state contributions: H[c, :] = U_c^T @ z_c accumulated into one
            # PSUM tile via the block-diagonal U slabs
            h_ps = hps.tile([C, DBLK], f32, tag="h")
            for c in range(C):
                nc.tensor.matmul(
                    h_ps[:],
                    Ublk[:, c * C : (c + 1) * C],
                    zall[:, c * DBLK : (c + 1) * DBLK],
                    start=(c == 0), stop=(c == C - 1),
                )
            H = sp.tile([C, DBLK], bf16, tag="H")
            nc.vector.tensor_copy(H[:], h_ps[:])

            c_ps = hps.tile([C, DBLK], f32, tag="cps")
            nc.tensor.matmul(c_ps[:], M2[:], H[:])
            carry = sp.tile([C, DBLK], bf16, tag="carry")
            nc.vector.tensor_copy(carry[:], c_ps[:])
            # flatten carry rows to one partition for the rank-1 matmuls
            cflat = sp.tile([1, C * DBLK], bf16, tag="cflat")
            nc.sync.dma_start(cflat[:], carry[:])

            # W^T blocks: block-diagonal K=12 delta matmul per group of 4
            # chunks (fp32r, N=512 -> 1 cycle/row), mask, exp
            wT = []
            for g in range(NG):
                dp = dps.tile([Q, GRP * Q], f32, tag="dp")
                nc.tensor.matmul(
                    dp[:],
                    sL[:, g * Q : (g + 1) * Q],
                    sR[:, g * GRP * Q : (g + 1) * GRP * Q],
                )
                nc.vector.tensor_add(dp[:], dp[:], mb[:])
                w4 = wp.tile([Q, GRP * Q], bf16, tag="w4")
                nc.scalar.activation(w4[:], dp[:], Exp)
                wT.append(w4)

            # outputs, two chunks per PSUM tile:
            # out_c = W_c^T.T @ z_c + A_c (x) carry_c
            osb = zp.tile([Q, FD], f32, tag="osb")
            for p in range(C // 2):
                o_ps = ops.tile([Q, 2 * DBLK], f32, tag="o")
                for h in range(2):
                    c = 2 * p + h
                    g, k = divmod(c, GRP)
                    dsl = slice(h * DBLK, (h + 1) * DBLK)
                    nc.tensor.matmul(
                        o_ps[:, dsl],
                        wT[g][:, k * Q : (k + 1) * Q],
                        zall[:, c * DBLK : (c + 1) * DBLK],
                        start=True, stop=False,
                    )
                    nc.tensor.matmul(
                        o_ps[:, dsl],
                        aflat[:, c * Q : (c + 1) * Q],
                        cflat[:, c * DBLK : (c + 1) * DBLK],
                        start=False, stop=True,
                    )
                osl = slice(2 * p * DBLK, (2 * p + 2) * DBLK)
                if p % 2 == 0:
                    nc.scalar.copy(osb[:, osl], o_ps[:])
                else:
                    nc.vector.tensor_copy(osb[:, osl], o_ps[:])

            ssl = FD // NZDMA
            for s in range(NZDMA):
                nc.sync.dma_start(
                    out_s[:, s * ssl : (s + 1) * ssl],
                    osb[:, s * ssl : (s + 1) * ssl],
                )

    nc.compile()
    return nc


def _host_prep(pt_b):
    """Per-batch host-side prep of the small scan operands. pt_b: [L] f32."""
    pt_b = pt_b.astype(np.float64)
    decay = np.clip(1.0 - pt_b, 1e-12, None)
    S = np.cumsum(np.log(decay))
    logp = np.log(np.maximum(pt_b, 1e-38))
    Send = S[Q - 1 :: Q]
    Sendprev = np.concatenate([[0.0], Send[:-1]])

    Sm = S.reshape(C, Q)
    logpm = logp.reshape(C, Q)
    # Re-center S within each chunk (see module docstring) and pre-round
    # operands to bf16-hi+lo representable values so the fp32r matmul
    # decomposition is exact.
    Sc = Sm - Sm[:, :1]

    def r16(x):
        h = x.astype(ml_dtypes.bfloat16).astype(np.float64)
        l = (x - h).astype(ml_dtypes.bfloat16).astype(np.float64)
        return h + l

    Sc = r16(Sc)
    logpr = r16(logpm)

    stackL = np.zeros((3 * GRP, NG * Q), np.float32)
    stackR = np.zeros((3 * GRP, NG * GRP * Q), np.float32)
    for g in range(NG):
        for k in range(GRP):
            c = g * GRP + k
            lcol = slice(g * Q, (g + 1) * Q)
            stackL[3 * k + 0, lcol] = 1.0
            stackL[3 * k + 1, lcol] = -Sc[c]
            stackL[3 * k + 2, lcol] = logpr[c]
            rcol = slice(g * GRP * Q + k * Q, g * GRP * Q + (k + 1) * Q)
            stackR[3 * k + 0, rcol] = Sc[c]
            stackR[3 * k + 1, rcol] = 1.0
            stackR[3 * k + 2, rcol] = 1.0

    # block-diagonal U exponent input: column block c holds
    # Send_c - S_j + logp_j in its own column c, NEG elsewhere
    uexp = (Send[:, None] - Sm + logpm).T  # [Q, C]
    uexpblk = np.full((Q, C * C), NEG, np.float32)
    for c in range(C):
        uexpblk[:, c * C + c] = uexp[:, c]

    aexp = (Sm - Sendprev[:, None]).astype(np.float32)
    m_i = np.arange(C)[:, None]
    c_i = np.arange(C)[None, :]
    d2exp = np.where(m_i < c_i, Sendprev[None, :] - Send[:, None], NEG)
    d2exp = d2exp.astype(np.float32)
    return stackL, stackR, uexpblk, aexp, d2exp


_MASKB = None


def _get_maskb():
    global _MASKB
    if _MASKB is None:
        j = np.arange(Q)[:, None]
        i = np.arange(Q)[None, :]
        one = np.where(i >= j, 0.0, NEG).astype(np.float32)
        _MASKB = np.tile(one, (1, GRP))
    return _MASKB


def _make_in_maps(z, pt):
    maskb = _get_maskb()
    preps = [_host_prep(pt[b]) for b in range(B)]
    in_maps = []
    for core in range(N_CORES):
        b, dblk = divmod(core, ND)
        stackL, stackR, uexpblk, aexp, d2exp = preps[b]
        z_slab = (
            z[b, :, dblk * DBLK : (dblk + 1) * DBLK]
            .reshape(C, Q, DBLK)
            .transpose(1, 0, 2)
            .reshape(Q, C * DBLK)
            .astype(ml_dtypes.bfloat16)
        )
        in_maps.append({
            "z_s": np.ascontiguousarray(z_slab),
            "stackL": stackL,
            "stackR": stackR,
            "uexpblk": uexpblk,
            "aexp": aexp,
            "d2exp": d2exp,
            "maskb": maskb,
        })
    return in_maps


def _unpack_out(res_core):
    """out_s [Q, C*DBLK] position-major -> [L, DBLK]."""
    return (
        res_core.reshape(Q, C, DBLK).transpose(1, 0, 2).reshape(L, DBLK)
    )


def _install_ntff_shim():
    """Enable NTFF profiling under axon: shim the missing antenv.axon_hooks
    module and register the ctypes hook from trn_boot; skip the fileshare
    artifact upload (no bucket in this container)."""
    import sys
    import types
    import antenv

    if "antenv.axon_hooks" not in sys.modules:
        mod = types.ModuleType("antenv.axon_hooks")
        hook_box = [None]
        mod.set_axon_ntff_profile_hook = lambda h: hook_box.__setitem__(0, h)
        mod.get_axon_ntff_profile_hook = lambda: hook_box[0]
        mod._hook_box = hook_box
        sys.modules["antenv.axon_hooks"] = mod
        antenv.axon_hooks = mod
    mod = sys.modules["antenv.axon_hooks"]
    if mod.get_axon_ntff_profile_hook() is None:
        from trn_agent_boot.trn_boot import _ntff_profile_via_ctypes

        mod.set_axon_ntff_profile_hook(
            _ntff_profile_via_ctypes("/opt/axon/libaxon_pjrt.so")
        )
    import concourse.bass_utils as bu

    bu.upload_artifacts = lambda tmpdir: f"local://{tmpdir}"


def kernel(z, pt):
    global LAST_EXEC_NS
    from concourse.bass_utils import run_bass_kernel_spmd

    z = np.asarray(z, dtype=np.float32)
    pt = np.asarray(pt, dtype=np.float32)

    if "nc" not in _CTX:
        _CTX["nc"] = _build_program()
    nc = _CTX["nc"]

    in_maps = _make_in_maps(z, pt)

    trace = bool(int(os.environ.get("BASS_KERNEL_TRACE", "0")))
    if trace:
        try:
            _install_ntff_shim()
        except Exception:
            trace = False
    tmpdir = os.environ.get("BASS_KERNEL_TRACE_DIR") or None
    res = run_bass_kernel_spmd(
        nc, in_maps, list(range(N_CORES)), trace=trace, tmpdir=tmpdir
    )
    LAST_EXEC_NS = res.exec_time_ns

    out = np.empty((B, L, D), np.float32)
    for core in range(N_CORES):
        b, dblk = divmod(core, ND)
        out[b, :, dblk * DBLK : (dblk + 1) * DBLK] = _unpack_out(
            res.results[core]["out_s"]
        )
    return out


# revision 19
# speedup vs baseline: 2.4585x; 1.0985x over previous
"""Trainium2 Bass kernel for DeChunking EMA (lower-triangular decay matmul).

Math: out[b,i,:] = sum_{j<=i} exp(S_i - S_j) * p_j * z[b,j,:],
with S = cumsum(log(clip(1-p))). Computed chunked-scan style (Mamba-SSD):

  - L split into C=32 chunks of Q=128.
  - Intra-chunk: out_intra = W_c^T.T @ z_c with
      W_c^T[j,i] = exp(S'_i - S'_j + log p_j) (masked to i>=j),
    where S' is S re-centered per chunk (only within-chunk differences
    matter, and small magnitudes survive the PE's fp32r mantissa split).
    The delta matrix is produced on PE by a block-diagonal stacked fp32r
    matmul: delta = 1*S'_i + (-S'_j)*1 + logp_j*1, K=3 per chunk -> K=12
    block-diagonal over a group of 4 chunks ([128,512] PSUM, 1 cycle/row).
  - Inter-chunk: chunk states H_c = U_c^T @ z_c accumulate into one
    [32,192] PSUM tile via a block-diagonal U (zero except column c of
    each [128,32] slab), then one [32,32] decay matmul forms all carry-in
    rows (carry = M2^T @ H), applied per chunk as a rank-1 PSUM-accumulated
    matmul out += A_c (x) carry_c.

All exp inputs are <= 0 by construction, so nothing overflows. The decay
weights / z / state operands run in bf16 on the PE (fp32 PSUM accumulate);
the delta stack runs in fp32r (S' re-centered + pre-rounded to bf16 hi+lo).

DRAM layouts are position-major ([Q, C*DBLK]) so every DMA moves >=3 KiB
contiguous per partition.

Sharding (8 cores, no collectives): core = (batch b in {0,1}) x (one of 4
D-blocks of 192). Each core reads z[b, :, blk] and pt[b] only.
"""

import os
import numpy as np
import ml_dtypes

B, L, D = 2, 4096, 768
Q = 128
C = L // Q           # 32 chunks
ND = 4               # D blocks per batch
DBLK = D // ND       # 192
GRP = 4              # chunks per delta/exp group
NG = C // GRP        # 8 groups
NEG = -3.0e38
N_CORES = 8
NZDMA = 4            # z-load / out-store DMA splits

_CTX = {}
LAST_EXEC_NS = None


def _build_program():
    import concourse.bacc as bacc
    import concourse.mybir as mybir
    from concourse import tile

    f32 = mybir.dt.float32
    f32r = mybir.dt.float32r
    bf16 = mybir.dt.bfloat16
    nc = bacc.Bacc("TRN2", target_bir_lowering=False, debug=False,
                   num_devices=N_CORES)

    FD = C * DBLK  # 6144 free elems in the big position-major tiles
    SLAB = C // NZDMA          # 8 chunks per z slab tile
    ZSL = SLAB * DBLK          # free elems per slab
    z_s = nc.dram_tensor("z_s", [Q, FD], bf16, kind="ExternalInput")
    stackL = nc.dram_tensor("stackL", [3 * GRP, NG * Q], f32r, kind="ExternalInput")
    stackR = nc.dram_tensor("stackR", [3 * GRP, NG * GRP * Q], f32r,
                            kind="ExternalInput")
    uexpblk = nc.dram_tensor("uexpblk", [Q, C * C], f32, kind="ExternalInput")
    d2exp = nc.dram_tensor("d2exp", [C, C], f32, kind="ExternalInput")
    kappa = nc.dram_tensor("kappa", [C, 1], f32, kind="ExternalInput")
    maskb = nc.dram_tensor("maskb", [Q, GRP * Q], f32, kind="ExternalInput")
    out_s = nc.dram_tensor("out_s", [Q, FD], f32, kind="ExternalOutput")

    Exp = mybir.ActivationFunctionType.Exp

    with tile.TileContext(nc) as tc:
        with (
            tc.tile_pool(name="zp", bufs=NZDMA) as zp,
            tc.tile_pool(name="wp", bufs=NG) as wp,
            tc.tile_pool(name="sp", bufs=1) as sp,
            tc.tile_pool(name="dps", bufs=3, space="PSUM") as dps,
            tc.tile_pool(name="ops", bufs=3, space="PSUM") as ops,
            tc.tile_pool(name="hps", bufs=1, space="PSUM") as hps,
        ):
            # sync-queue loads, smallest/most-urgent first (sL/sR feed the
            # delta matmuls that keep PE busy from the start)
            sL = sp.tile([3 * GRP, NG * Q], f32r, tag="sL")
            nc.sync.dma_start(sL[:], stackL[:])
            sR = sp.tile([3 * GRP, NG * GRP * Q], f32r, tag="sR")
            nc.sync.dma_start(sR[:], stackR[:])
            mb = sp.tile([Q, GRP * Q], f32, tag="mb")
            nc.sync.dma_start(mb[:], maskb[:])
            d2 = sp.tile([C, C], f32, tag="d2")
            nc.sync.dma_start(d2[:], d2exp[:])
            kp = sp.tile([C, 1], f32, tag="kp")
            nc.sync.dma_start(kp[:], kappa[:])
            ub = sp.tile([Q, C * C], f32, tag="ub")
            nc.sync.dma_start(ub[:], uexpblk[:])

            # z slabs on the gpsimd queue so they start immediately and
            # never wait behind dependent sync-queue DMAs
            zsl = []
            for s in range(NZDMA):
                t = zp.tile([Q, ZSL], bf16, tag="z")
                nc.gpsimd.dma_start(t[:], z_s[:, s * ZSL : (s + 1) * ZSL])
                zsl.append(t)

            def zchunk(c):
                s, r = divmod(c, SLAB)
                return zsl[s][:, r * DBLK : (r + 1) * DBLK]

            # PE from the top: the W^T delta matmuls only need sL/sR.
            # Block-diagonal K=12 fp32r matmul per group of 4 chunks
            # (N=512 -> 1 cycle/row), then mask add, then exp.
            wT = []
            for g in range(NG):
                dp = dps.tile([Q, GRP * Q], f32, tag="dp")
                nc.tensor.matmul(
                    dp[:],
                    sL[:, g * Q : (g + 1) * Q],
                    sR[:, g * GRP * Q : (g + 1) * GRP * Q],
                )
                nc.vector.tensor_add(dp[:], dp[:], mb[:])
                w4 = wp.tile([Q, GRP * Q], bf16, tag="w4")
                nc.scalar.activation(w4[:], dp[:], Exp)
                wT.append(w4)

            Ublk = sp.tile([Q, C * C], bf16, tag="Ublk")
            nc.scalar.activation(Ublk[:], ub[:], Exp)
            M2 = sp.tile([C, C], bf16, tag="M2")
            nc.scalar.activation(M2[:], d2[:], Exp)

            # state contributions: H[c, :] = U_c^T @ z_c accumulated into one
            # PSUM tile via the block-diagonal U slabs
            h_ps = hps.tile([C, DBLK], f32, tag="h")
            for c in range(C):
                nc.tensor.matmul(
                    h_ps[:],
                    Ublk[:, c * C : (c + 1) * C],
                    zchunk(c),
                    start=(c == 0), stop=(c == C - 1),
                )
            H = sp.tile([C, DBLK], bf16, tag="H")
            nc.vector.tensor_copy(H[:], h_ps[:])

            c_ps = hps.tile([C, DBLK], f32, tag="cps")
            nc.tensor.matmul(c_ps[:], M2[:], H[:])
            # scale carry rows by kappa so the whole carry application can be
            # folded into row 0 of z (out += a (x) carry == W^T row 0 doing
            # the rank-1 update once z[0] += kappa*carry)
            cfk = sp.tile([C, DBLK], bf16, tag="cfk")
            nc.vector.tensor_scalar_mul(cfk[:], c_ps[:], kp[:])
            for s in range(NZDMA):
                nc.gpsimd.dma_start(
                    zsl[s][0:1, :],
                    cfk[s * SLAB : (s + 1) * SLAB, :],
                    accum_op=mybir.AluOpType.add,
                )

            # outputs, two chunks per PSUM tile: out_c = W_c'^T.T @ z'_c
            osb = sp.tile([Q, FD], f32, tag="osb")
            for p in range(C // 2):
                o_ps = ops.tile([Q, 2 * DBLK], f32, tag="o")
                for h in range(2):
                    c = 2 * p + h
                    g, k = divmod(c, GRP)
                    nc.tensor.matmul(
                        o_ps[:, h * DBLK : (h + 1) * DBLK],
                        wT[g][:, k * Q : (k + 1) * Q],
                        zchunk(c),
                    )
                osl = slice(2 * p * DBLK, (2 * p + 2) * DBLK)
                if p % 2 == 0:
                    nc.scalar.copy(osb[:, osl], o_ps[:])
                else:
                    nc.vector.tensor_copy(osb[:, osl], o_ps[:])

            ssl = FD // NZDMA
            for s in range(NZDMA):
                nc.sync.dma_start(
                    out_s[:, s * ssl : (s + 1) * ssl],
                    osb[:, s * ssl : (s + 1) * ssl],
                )

    nc.compile()
    return nc


def _host_prep(pt_b):
    """Per-batch host-side prep of the small scan operands. pt_b: [L] f32."""
    pt_b = pt_b.astype(np.float64)
    decay = np.clip(1.0 - pt_b, 1e-12, None)
    S = np.cumsum(np.log(decay))
    logp = np.log(np.maximum(pt_b, 1e-38))
    Send = S[Q - 1 :: Q]
    Sendprev = np.concatenate([[0.0], Send[:-1]])

    Sm = S.reshape(C, Q)
    logpm = logp.reshape(C, Q)
    # Re-center S within each chunk (see module docstring) and pre-round
    # operands to bf16-hi+lo representable values so the fp32r matmul
    # decomposition is exact.
    Sc = Sm - Sm[:, :1]

    def r16(x):
        h = x.astype(ml_dtypes.bfloat16).astype(np.float64)
        l = (x - h).astype(ml_dtypes.bfloat16).astype(np.float64)
        return h + l

    Sc = r16(Sc)
    logpr = r16(logpm)

    stackL = np.zeros((3 * GRP, NG * Q), np.float32)
    stackR = np.zeros((3 * GRP, NG * GRP * Q), np.float32)
    for g in range(NG):
        for k in range(GRP):
            c = g * GRP + k
            lcol = slice(g * Q, (g + 1) * Q)
            stackL[3 * k + 0, lcol] = 1.0
            stackL[3 * k + 1, lcol] = -Sc[c]
            stackL[3 * k + 2, lcol] = logpr[c]
            rcol = slice(g * GRP * Q + k * Q, g * GRP * Q + (k + 1) * Q)
            stackR[3 * k + 0, rcol] = Sc[c]
            stackR[3 * k + 1, rcol] = 1.0
            stackR[3 * k + 2, rcol] = 1.0

    # block-diagonal U exponent input: column block c holds
    # Send_c - S_j + logp_j in its own column c, NEG elsewhere
    uexp = (Send[:, None] - Sm + logpm).T  # [Q, C]
    uexpblk = np.full((Q, C * C), NEG, np.float32)
    for c in range(C):
        uexpblk[:, c * C + c] = uexp[:, c]

    m_i = np.arange(C)[:, None]
    c_i = np.arange(C)[None, :]
    d2exp = np.where(m_i < c_i, Sendprev[None, :] - Send[:, None], NEG)
    d2exp = d2exp.astype(np.float32)

    # kappa_c = exp(S_{c,0} - Send_{c-1} - logp_r[c,0]): scaling such that
    # W^T row 0 (= exp(S'_i + logp_r[c,0])) times kappa*carry reproduces the
    # rank-1 carry term a_i*carry. Uses the device-rounded logp so the
    # coefficient reconstruction cancels exactly.
    kap = np.exp(np.minimum(Sm[:, 0] - Sendprev - logpr[:, 0], 69.0))
    kappa = kap.reshape(C, 1).astype(np.float32)
    return stackL, stackR, uexpblk, d2exp, kappa


_MASKB = None


def _get_maskb():
    global _MASKB
    if _MASKB is None:
        j = np.arange(Q)[:, None]
        i = np.arange(Q)[None, :]
        one = np.where(i >= j, 0.0, NEG).astype(np.float32)
        _MASKB = np.tile(one, (1, GRP))
    return _MASKB


def _make_in_maps(z, pt):
    maskb = _get_maskb()
    preps = [_host_prep(pt[b]) for b in range(B)]
    in_maps = []
    for core in range(N_CORES):
        b, dblk = divmod(core, ND)
        stackL, stackR, uexpblk, d2exp, kappa = preps[b]
        z_slab = (
            z[b, :, dblk * DBLK : (dblk + 1) * DBLK]
            .reshape(C, Q, DBLK)
            .transpose(1, 0, 2)
            .reshape(Q, C * DBLK)
            .astype(ml_dtypes.bfloat16)
        )
        in_maps.append({
            "z_s": np.ascontiguousarray(z_slab),
            "stackL": stackL,
            "stackR": stackR,
            "uexpblk": uexpblk,
            "d2exp": d2exp,
            "kappa": kappa,
            "maskb": maskb,
        })
    return in_maps


def _unpack_out(res_core):
    """out_s [Q, C*DBLK] position-major -> [L, DBLK]."""
    return (
        res_core.reshape(Q, C, DBLK).transpose(1, 0, 2).reshape(L, DBLK)
    )


def _install_ntff_shim():
    """Enable NTFF profiling under axon: shim the missing antenv.axon_hooks
    module and register the ctypes hook from trn_boot; skip the fileshare
    artifact upload (no bucket in this container)."""
    import sys
    import types
    import antenv

    if "antenv.axon_hooks" not in sys.modules:
        mod = types.ModuleType("antenv.axon_hooks")
        hook_box = [None]
        mod.set_axon_ntff_profile_hook = lambda h: hook_box.__setitem__(0, h)
        mod.get_axon_ntff_profile_hook = lambda: hook_box[0]
        mod._hook_box = hook_box
        sys.modules["antenv.axon_hooks"] = mod
        antenv.axon_hooks = mod
    mod = sys.modules["antenv.axon_hooks"]
    if mod.get_axon_ntff_profile_hook() is None:
        from trn_agent_boot.trn_boot import _ntff_profile_via_ctypes

        mod.set_axon_ntff_profile_hook(
            _ntff_profile_via_ctypes("/opt/axon/libaxon_pjrt.so")
        )
    import concourse.bass_utils as bu

    bu.upload_artifacts = lambda tmpdir: f"local://{tmpdir}"


def kernel(z, pt):
    global LAST_EXEC_NS
    from concourse.bass_utils import run_bass_kernel_spmd

    z = np.asarray(z, dtype=np.float32)
    pt = np.asarray(pt, dtype=np.float32)

    if "nc" not in _CTX:
        _CTX["nc"] = _build_program()
    nc = _CTX["nc"]

    in_maps = _make_in_maps(z, pt)

    trace = bool(int(os.environ.get("BASS_KERNEL_TRACE", "0")))
    if trace:
        try:
            _install_ntff_shim()
        except Exception:
            trace = False
    tmpdir = os.environ.get("BASS_KERNEL_TRACE_DIR") or None
    res = run_bass_kernel_spmd(
        nc, in_maps, list(range(N_CORES)), trace=trace, tmpdir=tmpdir
    )
    LAST_EXEC_NS = res.exec_time_ns

    out = np.empty((B, L, D), np.float32)
    for core in range(N_CORES):
        b, dblk = divmod(core, ND)
        out[b, :, dblk * DBLK : (dblk + 1) * DBLK] = _unpack_out(
            res.results[core]["out_s"]
        )
    return out


# revision 22
# speedup vs baseline: 2.4713x; 1.0052x over previous
"""Trainium2 Bass kernel for DeChunking EMA (lower-triangular decay matmul).

Math: out[b,i,:] = sum_{j<=i} exp(S_i - S_j) * p_j * z[b,j,:],
with S = cumsum(log(clip(1-p))). Computed chunked-scan style (Mamba-SSD):

  - L split into C=32 chunks of Q=128.
  - Intra-chunk: out_intra = W_c^T.T @ z_c with
      W_c^T[j,i] = exp(S'_i - S'_j + log p_j) (masked to i>=j),
    where S' is S re-centered per chunk (only within-chunk differences
    matter, and small magnitudes survive the PE's fp32r mantissa split).
    The delta matrix is produced on PE by a block-diagonal stacked fp32r
    matmul: delta = 1*S'_i + (-S'_j)*1 + logp_j*1, K=3 per chunk -> K=12
    block-diagonal over a group of 4 chunks ([128,512] PSUM, 1 cycle/row).
  - Inter-chunk: chunk states H_c = U_c^T @ z_c accumulate into one
    [32,192] PSUM tile via a block-diagonal U (zero except column c of
    each [128,32] slab), then one [32,32] decay matmul forms all carry-in
    rows (carry = M2^T @ H), applied per chunk as a rank-1 PSUM-accumulated
    matmul out += A_c (x) carry_c.

All exp inputs are <= 0 by construction, so nothing overflows. The decay
weights / z / state operands run in bf16 on the PE (fp32 PSUM accumulate);
the delta stack runs in fp32r (S' re-centered + pre-rounded to bf16 hi+lo).

DRAM layouts are position-major ([Q, C*DBLK]) so every DMA moves >=3 KiB
contiguous per partition.

Sharding (8 cores, no collectives): core = (batch b in {0,1}) x (one of 4
D-blocks of 192). Each core reads z[b, :, blk] and pt[b] only.
"""

import os
import numpy as np
import ml_dtypes

B, L, D = 2, 4096, 768
Q = 128
C = L // Q           # 32 chunks
ND = 4               # D blocks per batch
DBLK = D // ND       # 192
GRP = 4              # chunks per delta/exp group
NG = C // GRP        # 8 groups
NEG = -3.0e38
N_CORES = 8
NZDMA = 4            # z-load / out-store DMA splits

_CTX = {}
LAST_EXEC_NS = None


def _build_program():
    import concourse.bacc as bacc
    import concourse.mybir as mybir
    from concourse import tile

    f32 = mybir.dt.float32
    f32r = mybir.dt.float32r
    bf16 = mybir.dt.bfloat16
    nc = bacc.Bacc("TRN2", target_bir_lowering=False, debug=False,
                   num_devices=N_CORES)

    FD = C * DBLK  # 6144 free elems in the big position-major tiles
    SLAB = C // NZDMA          # 8 chunks per z slab tile
    ZSL = SLAB * DBLK          # free elems per slab
    z_s = nc.dram_tensor("z_s", [Q, FD], bf16, kind="ExternalInput")
    # packed aux inputs: one DMA per partition-count class
    aux12 = nc.dram_tensor("aux12", [3 * GRP, NG * Q + NG * GRP * Q], f32r,
                           kind="ExternalInput")
    aux128 = nc.dram_tensor("aux128", [Q, GRP * Q + C], f32,
                            kind="ExternalInput")
    aux32 = nc.dram_tensor("aux32", [C, C + 1], f32, kind="ExternalInput")
    out_s = nc.dram_tensor("out_s", [Q, FD], f32, kind="ExternalOutput")

    Exp = mybir.ActivationFunctionType.Exp

    with tile.TileContext(nc) as tc:
        with (
            tc.tile_pool(name="zp", bufs=NZDMA) as zp,
            tc.tile_pool(name="wp", bufs=NG) as wp,
            tc.tile_pool(name="sp", bufs=1) as sp,
            tc.tile_pool(name="dps", bufs=3, space="PSUM") as dps,
            tc.tile_pool(name="ops", bufs=3, space="PSUM") as ops,
            tc.tile_pool(name="hps", bufs=1, space="PSUM") as hps,
        ):
            # z slabs stream on the sync HWDGE queue from the very top
            zsl = []
            for s in range(NZDMA):
                t = zp.tile([Q, ZSL], bf16, tag="z")
                nc.sync.dma_start(t[:], z_s[:, s * ZSL : (s + 1) * ZSL])
                zsl.append(t)

            def zchunk(c):
                s, r = divmod(c, SLAB)
                return zsl[s][:, r * DBLK : (r + 1) * DBLK]

            # aux loads ride the Activation HWDGE queue in parallel
            a12 = sp.tile([3 * GRP, NG * Q + NG * GRP * Q], f32r, tag="a12")
            nc.scalar.dma_start(a12[:], aux12[:])
            a128 = sp.tile([Q, GRP * Q + C], f32, tag="a128")
            nc.scalar.dma_start(a128[:], aux128[:])
            a32 = sp.tile([C, C + 1], f32, tag="a32")
            nc.scalar.dma_start(a32[:], aux32[:])
            sL = a12[:, 0 : NG * Q]
            sR = a12[:, NG * Q :]
            mb = a128[:, 0 : GRP * Q]
            ue = a128[:, GRP * Q :]
            d2 = a32[:, 0:C]
            kp = a32[:, C : C + 1]

            # U block-diagonal [Q, C*C] bf16: zero it, exp the [Q, C] column
            # stack, scatter onto the diagonal (stride C+1 in the free dim)
            Ublk = sp.tile([Q, C * C], bf16, tag="Ublk")
            nc.gpsimd.memset(Ublk[:], 0.0)
            Us = sp.tile([Q, C], bf16, tag="Us")
            nc.scalar.activation(Us[:], ue, Exp)
            nc.scalar.copy(Ublk[:, 0 : C * C : C + 1], Us[:])
            M2 = sp.tile([C, C], bf16, tag="M2")
            nc.scalar.activation(M2[:], d2, Exp)

            # PE: W^T delta matmuls (block-diagonal K=12 fp32r, N=512 -> 1
            # cycle/row) interleaved with the H state matmuls as z lands
            wT = []
            h_ps = hps.tile([C, DBLK], f32, tag="h")

            def delta_group(g):
                dp = dps.tile([Q, GRP * Q], f32, tag="dp")
                nc.tensor.matmul(
                    dp[:],
                    sL[:, g * Q : (g + 1) * Q],
                    sR[:, g * GRP * Q : (g + 1) * GRP * Q],
                )
                nc.vector.tensor_add(dp[:], dp[:], mb)
                w4 = wp.tile([Q, GRP * Q], bf16, tag="w4")
                nc.scalar.activation(w4[:], dp[:], Exp)
                wT.append(w4)

            def h_slab(s):
                for r in range(SLAB):
                    c = s * SLAB + r
                    nc.tensor.matmul(
                        h_ps[:],
                        Ublk[:, c * C : (c + 1) * C],
                        zchunk(c),
                        start=(c == 0), stop=(c == C - 1),
                        skip_group_check=True,
                    )

            for s in range(NZDMA):
                delta_group(2 * s)
                delta_group(2 * s + 1)
                h_slab(s)

            H = sp.tile([C, DBLK], bf16, tag="H")
            nc.vector.tensor_copy(H[:], h_ps[:])
            c_ps = hps.tile([C, DBLK], f32, tag="cps")
            nc.tensor.matmul(c_ps[:], M2[:], H[:])
            # scale carry rows by kappa so the whole carry application can be
            # folded into row 0 of z (out += a (x) carry == W^T row 0 doing
            # the rank-1 update once z[0] += kappa*carry)
            cfk = sp.tile([C, DBLK], bf16, tag="cfk")
            nc.vector.tensor_scalar_mul(cfk[:], c_ps[:], kp)
            for s in range(NZDMA):
                nc.gpsimd.dma_start(
                    zsl[s][0:1, :],
                    cfk[s * SLAB : (s + 1) * SLAB, :],
                    accum_op=mybir.AluOpType.add,
                )

            # outputs, two chunks per PSUM tile: out_c = W_c'^T.T @ z'_c
            osb = sp.tile([Q, FD], f32, tag="osb")
            for p in range(C // 2):
                o_ps = ops.tile([Q, 2 * DBLK], f32, tag="o")
                for h in range(2):
                    c = 2 * p + h
                    g, k = divmod(c, GRP)
                    nc.tensor.matmul(
                        o_ps[:, h * DBLK : (h + 1) * DBLK],
                        wT[g][:, k * Q : (k + 1) * Q],
                        zchunk(c),
                    )
                osl = slice(2 * p * DBLK, (2 * p + 2) * DBLK)
                if p % 2 == 0:
                    nc.scalar.copy(osb[:, osl], o_ps[:])
                else:
                    nc.vector.tensor_copy(osb[:, osl], o_ps[:])

            ssl = FD // NZDMA
            for s in range(NZDMA):
                nc.sync.dma_start(
                    out_s[:, s * ssl : (s + 1) * ssl],
                    osb[:, s * ssl : (s + 1) * ssl],
                )

    nc.compile()
    return nc


def _host_prep(pt_b):
    """Per-batch host-side prep of the small scan operands. pt_b: [L] f32."""
    pt_b = pt_b.astype(np.float64)
    decay = np.clip(1.0 - pt_b, 1e-12, None)
    S = np.cumsum(np.log(decay))
    logp = np.log(np.maximum(pt_b, 1e-38))
    Send = S[Q - 1 :: Q]
    Sendprev = np.concatenate([[0.0], Send[:-1]])

    Sm = S.reshape(C, Q)
    logpm = logp.reshape(C, Q)
    # Re-center S within each chunk (see module docstring) and pre-round
    # operands to bf16-hi+lo representable values so the fp32r matmul
    # decomposition is exact.
    Sc = Sm - Sm[:, :1]

    def r16(x):
        h = x.astype(ml_dtypes.bfloat16).astype(np.float64)
        l = (x - h).astype(ml_dtypes.bfloat16).astype(np.float64)
        return h + l

    Sc = r16(Sc)
    logpr = r16(logpm)

    stackL = np.zeros((3 * GRP, NG * Q), np.float32)
    stackR = np.zeros((3 * GRP, NG * GRP * Q), np.float32)
    for g in range(NG):
        for k in range(GRP):
            c = g * GRP + k
            lcol = slice(g * Q, (g + 1) * Q)
            stackL[3 * k + 0, lcol] = 1.0
            stackL[3 * k + 1, lcol] = -Sc[c]
            stackL[3 * k + 2, lcol] = logpr[c]
            rcol = slice(g * GRP * Q + k * Q, g * GRP * Q + (k + 1) * Q)
            stackR[3 * k + 0, rcol] = Sc[c]
            stackR[3 * k + 1, rcol] = 1.0
            stackR[3 * k + 2, rcol] = 1.0

    # U exponent column stack: Send_c - S_j + logp_j  -> [Q, C]
    uexp = (Send[:, None] - Sm + logpm).T.astype(np.float32)

    m_i = np.arange(C)[:, None]
    c_i = np.arange(C)[None, :]
    d2exp = np.where(m_i < c_i, Sendprev[None, :] - Send[:, None], NEG)
    d2exp = d2exp.astype(np.float32)

    # kappa_c = exp(S_{c,0} - Send_{c-1} - logp_r[c,0]): scaling such that
    # W^T row 0 (= exp(S'_i + logp_r[c,0])) times kappa*carry reproduces the
    # rank-1 carry term a_i*carry. Uses the device-rounded logp so the
    # coefficient reconstruction cancels exactly.
    kap = np.exp(np.minimum(Sm[:, 0] - Sendprev - logpr[:, 0], 69.0))
    kappa = kap.reshape(C, 1).astype(np.float32)

    aux12 = np.concatenate([stackL, stackR], axis=1)
    aux128 = np.concatenate([_get_maskb(), uexp], axis=1)
    aux32 = np.concatenate([d2exp, kappa], axis=1)
    return aux12, aux128, aux32


_MASKB = None


def _get_maskb():
    global _MASKB
    if _MASKB is None:
        j = np.arange(Q)[:, None]
        i = np.arange(Q)[None, :]
        one = np.where(i >= j, 0.0, NEG).astype(np.float32)
        _MASKB = np.tile(one, (1, GRP))
    return _MASKB


def _make_in_maps(z, pt):
    preps = [_host_prep(pt[b]) for b in range(B)]
    in_maps = []
    for core in range(N_CORES):
        b, dblk = divmod(core, ND)
        aux12, aux128, aux32 = preps[b]
        z_slab = (
            z[b, :, dblk * DBLK : (dblk + 1) * DBLK]
            .reshape(C, Q, DBLK)
            .transpose(1, 0, 2)
            .reshape(Q, C * DBLK)
            .astype(ml_dtypes.bfloat16)
        )
        in_maps.append({
            "z_s": np.ascontiguousarray(z_slab),
            "aux12": aux12,
            "aux128": aux128,
            "aux32": aux32,
        })
    return in_maps


def _unpack_out(res_core):
    """out_s [Q, C*DBLK] position-major -> [L, DBLK]."""
    return (
        res_core.reshape(Q, C, DBLK).transpose(1, 0, 2).reshape(L, DBLK)
    )


def _install_ntff_shim():
    """Enable NTFF profiling under axon: shim the missing antenv.axon_hooks
    module and register the ctypes hook from trn_boot; skip the fileshare
    artifact upload (no bucket in this container)."""
    import sys
    import types
    import antenv

    if "antenv.axon_hooks" not in sys.modules:
        mod = types.ModuleType("antenv.axon_hooks")
        hook_box = [None]
        mod.set_axon_ntff_profile_hook = lambda h: hook_box.__setitem__(0, h)
        mod.get_axon_ntff_profile_hook = lambda: hook_box[0]
        mod._hook_box = hook_box
        sys.modules["antenv.axon_hooks"] = mod
        antenv.axon_hooks = mod
    mod = sys.modules["antenv.axon_hooks"]
    if mod.get_axon_ntff_profile_hook() is None:
        from trn_agent_boot.trn_boot import _ntff_profile_via_ctypes

        mod.set_axon_ntff_profile_hook(
            _ntff_profile_via_ctypes("/opt/axon/libaxon_pjrt.so")
        )
    import concourse.bass_utils as bu

    bu.upload_artifacts = lambda tmpdir: f"local://{tmpdir}"


def kernel(z, pt):
    global LAST_EXEC_NS
    from concourse.bass_utils import run_bass_kernel_spmd

    z = np.asarray(z, dtype=np.float32)
    pt = np.asarray(pt, dtype=np.float32)

    if "nc" not in _CTX:
        _CTX["nc"] = _build_program()
    nc = _CTX["nc"]

    in_maps = _make_in_maps(z, pt)

    trace = bool(int(os.environ.get("BASS_KERNEL_TRACE", "0")))
    if trace:
        try:
            _install_ntff_shim()
        except Exception:
            trace = False
    tmpdir = os.environ.get("BASS_KERNEL_TRACE_DIR") or None
    res = run_bass_kernel_spmd(
        nc, in_maps, list(range(N_CORES)), trace=trace, tmpdir=tmpdir
    )
    LAST_EXEC_NS = res.exec_time_ns

    out = np.empty((B, L, D), np.float32)
    for core in range(N_CORES):
        b, dblk = divmod(core, ND)
        out[b, :, dblk * DBLK : (dblk + 1) * DBLK] = _unpack_out(
            res.results[core]["out_s"]
        )
    return out


# revision 27
# speedup vs baseline: 2.6520x; 1.0731x over previous
"""Trainium2 Bass kernel for DeChunking EMA (lower-triangular decay matmul).

Math: out[b,i,:] = sum_{j<=i} exp(S_i - S_j) * p_j * z[b,j,:],
with S = cumsum(log(clip(1-p))). Computed chunked-scan style (Mamba-SSD):

  - L split into C=32 chunks of Q=128.
  - Intra-chunk: out_intra = W_c^T.T @ z_c with
      W_c^T[j,i] = exp(S'_i - S'_j + log p_j) (masked to i>=j),
    where S' is S re-centered per chunk (only within-chunk differences
    matter, and small magnitudes survive the PE's fp32r mantissa split).
    The delta matrix is produced on PE by a block-diagonal stacked fp32r
    matmul: delta = 1*S'_i + (-S'_j)*1 + logp_j*1, K=3 per chunk -> K=12
    block-diagonal over a group of 4 chunks ([128,512] PSUM, 1 cycle/row).
  - Inter-chunk: chunk states H_c = U_c^T @ z_c accumulate into one
    [32,192] PSUM tile via a block-diagonal U (zero except column c of
    each [128,32] slab), then one [32,32] decay matmul forms all carry-in
    rows (carry = M2^T @ H), applied per chunk as a rank-1 PSUM-accumulated
    matmul out += A_c (x) carry_c.

All exp inputs are <= 0 by construction, so nothing overflows. The decay
weights / z / state operands run in bf16 on the PE (fp32 PSUM accumulate);
the delta stack runs in fp32r (S' re-centered + pre-rounded to bf16 hi+lo).

DRAM layouts are position-major ([Q, C*DBLK]) so every DMA moves >=3 KiB
contiguous per partition.

Sharding (8 cores, no collectives): core = (batch b in {0,1}) x (one of 4
D-blocks of 192). Each core reads z[b, :, blk] and pt[b] only.
"""

import os
import numpy as np
import ml_dtypes

B, L, D = 2, 4096, 768
Q = 128
C = L // Q           # 32 chunks
ND = 4               # D blocks per batch
DBLK = D // ND       # 192
GRP = 4              # chunks per delta/exp group
NG = C // GRP        # 8 groups
NEG = -3.0e38
N_CORES = 8
NZDMA = 4            # z-load / out-store DMA splits

_CTX = {}
LAST_EXEC_NS = None


def _build_program():
    import concourse.bacc as bacc
    import concourse.mybir as mybir
    from concourse import tile

    f32 = mybir.dt.float32
    f32r = mybir.dt.float32r
    bf16 = mybir.dt.bfloat16
    nc = bacc.Bacc("TRN2", target_bir_lowering=False, debug=False,
                   num_devices=N_CORES)

    FD = C * DBLK  # 6144 free elems in the big position-major tiles
    SLAB = C // NZDMA          # 8 chunks per z slab tile
    ZSL = SLAB * DBLK          # free elems per slab
    z_s = nc.dram_tensor("z_s", [Q, FD], bf16, kind="ExternalInput")
    # packed aux inputs: one DMA per partition-count class
    aux12 = nc.dram_tensor("aux12", [3 * GRP, NG * Q + NG * GRP * Q], f32r,
                           kind="ExternalInput")
    aux128 = nc.dram_tensor("aux128", [Q, GRP * Q + C], f32,
                            kind="ExternalInput")
    aux32 = nc.dram_tensor("aux32", [C, C + 1], f32, kind="ExternalInput")
    out_s = nc.dram_tensor("out_s", [Q, FD], f32, kind="ExternalOutput")

    Exp = mybir.ActivationFunctionType.Exp

    with tile.TileContext(nc) as tc:
        with (
            tc.tile_pool(name="zp", bufs=NZDMA) as zp,
            tc.tile_pool(name="wp", bufs=NG) as wp,
            tc.tile_pool(name="sp", bufs=1) as sp,
            tc.tile_pool(name="dps", bufs=3, space="PSUM") as dps,
            tc.tile_pool(name="ops", bufs=2, space="PSUM") as ops,
            tc.tile_pool(name="hps", bufs=1, space="PSUM") as hps,
            tc.tile_pool(name="wps", bufs=1, space="PSUM") as wps,
        ):
            # aux12 gates the delta matmuls that start the PE pipeline: load
            # it first on sync, then stream the z slabs behind it
            a12 = sp.tile([3 * GRP, NG * Q + NG * GRP * Q], f32r, tag="a12")
            nc.sync.dma_start(a12[:], aux12[:])
            zsl = []
            for s in range(NZDMA):
                t = zp.tile([Q, ZSL], bf16, tag="z")
                nc.sync.dma_start(t[:], z_s[:, s * ZSL : (s + 1) * ZSL])
                zsl.append(t)

            def zchunk(c):
                s, r = divmod(c, SLAB)
                return zsl[s][:, r * DBLK : (r + 1) * DBLK]

            # remaining aux loads ride the Activation HWDGE queue in parallel
            a128 = sp.tile([Q, GRP * Q + C], f32, tag="a128")
            nc.scalar.dma_start(a128[:], aux128[:])
            a32 = sp.tile([C, C + 1], f32, tag="a32")
            nc.scalar.dma_start(a32[:], aux32[:])
            sL = a12[:, 0 : NG * Q]
            sR = a12[:, NG * Q :]
            mb = a128[:, 0 : GRP * Q]
            ue = a128[:, GRP * Q :]
            d2 = a32[:, 0:C]
            kp = a32[:, C : C + 1]

            # U block-diagonal [Q, C*C] bf16: zero it, exp the [Q, C] column
            # stack, scatter onto the diagonal (stride C+1 in the free dim)
            Ublk = sp.tile([Q, C * C], bf16, tag="Ublk")
            nc.gpsimd.memset(Ublk[:], 0.0)
            Us = sp.tile([Q, C], bf16, tag="Us")
            nc.scalar.activation(Us[:], ue, Exp)
            nc.scalar.copy(Ublk[:, 0 : C * C : C + 1], Us[:])
            M2 = sp.tile([C, C], bf16, tag="M2")
            nc.scalar.activation(M2[:], d2, Exp)

            # PE clock warmup: ~3.5us of back-to-back dense matmuls on junk
            # data during the input-DMA window flips the HAM gate to 2.4 GHz
            # before the real work arrives (the real matmuls are too sparse
            # in array-duty to flip it themselves)
            wm_sb = sp.tile([Q, 2 * Q], bf16, tag="wm_sb")
            nc.gpsimd.memset(wm_sb[:], 1.0)
            wm_ps = wps.tile([Q, 2 * Q], f32, tag="wm")
            for _ in range(14):
                nc.tensor.matmul(wm_ps[:], wm_sb[:, 0:Q], wm_sb[:])

            # PE: W^T delta matmuls (block-diagonal K=12 fp32r, N=512 -> 1
            # cycle/row) interleaved with the H state matmuls as z lands
            wT = []
            h_ps = hps.tile([C, DBLK], f32, tag="h")

            def delta_group(g):
                dp = dps.tile([Q, GRP * Q], f32, tag="dp")
                nc.tensor.matmul(
                    dp[:],
                    sL[:, g * Q : (g + 1) * Q],
                    sR[:, g * GRP * Q : (g + 1) * GRP * Q],
                )
                nc.vector.tensor_add(dp[:], dp[:], mb)
                w4 = wp.tile([Q, GRP * Q], bf16, tag="w4")
                nc.scalar.activation(w4[:], dp[:], Exp)
                wT.append(w4)

            def h_slab(s):
                for r in range(SLAB):
                    c = s * SLAB + r
                    nc.tensor.matmul(
                        h_ps[:],
                        Ublk[:, c * C : (c + 1) * C],
                        zchunk(c),
                        start=(c == 0), stop=(c == C - 1),
                        skip_group_check=True,
                    )

            for s in range(NZDMA):
                delta_group(2 * s)
                delta_group(2 * s + 1)
                h_slab(s)

            H = sp.tile([C, DBLK], bf16, tag="H")
            nc.vector.tensor_copy(H[:], h_ps[:])
            c_ps = hps.tile([C, DBLK], f32, tag="cps")
            nc.tensor.matmul(c_ps[:], M2[:], H[:])
            # scale carry rows by kappa so the whole carry application can be
            # folded into row 0 of z (out += a (x) carry == W^T row 0 doing
            # the rank-1 update once z[0] += kappa*carry)
            cfk = sp.tile([C, DBLK], bf16, tag="cfk")
            nc.vector.tensor_scalar_mul(cfk[:], c_ps[:], kp)
            for s in range(NZDMA):
                nc.gpsimd.dma_start(
                    zsl[s][0:1, :],
                    cfk[s * SLAB : (s + 1) * SLAB, :],
                    accum_op=mybir.AluOpType.add,
                )

            # outputs, two chunks per PSUM tile: out_c = W_c'^T.T @ z'_c
            osb = sp.tile([Q, FD], f32, tag="osb")
            for p in range(C // 2):
                o_ps = ops.tile([Q, 2 * DBLK], f32, tag="o")
                for h in range(2):
                    c = 2 * p + h
                    g, k = divmod(c, GRP)
                    nc.tensor.matmul(
                        o_ps[:, h * DBLK : (h + 1) * DBLK],
                        wT[g][:, k * Q : (k + 1) * Q],
                        zchunk(c),
                    )
                osl = slice(2 * p * DBLK, (2 * p + 2) * DBLK)
                if p % 2 == 0:
                    nc.scalar.copy(osb[:, osl], o_ps[:])
                else:
                    nc.vector.tensor_copy(osb[:, osl], o_ps[:])

            ssl = FD // NZDMA
            for s in range(NZDMA):
                nc.sync.dma_start(
                    out_s[:, s * ssl : (s + 1) * ssl],
                    osb[:, s * ssl : (s + 1) * ssl],
                )

    nc.compile()
    return nc


def _host_prep(pt_b):
    """Per-batch host-side prep of the small scan operands. pt_b: [L] f32."""
    pt_b = pt_b.astype(np.float64)
    decay = np.clip(1.0 - pt_b, 1e-12, None)
    S = np.cumsum(np.log(decay))
    logp = np.log(np.maximum(pt_b, 1e-38))
    Send = S[Q - 1 :: Q]
    Sendprev = np.concatenate([[0.0], Send[:-1]])

    Sm = S.reshape(C, Q)
    logpm = logp.reshape(C, Q)
    # Re-center S within each chunk (see module docstring) and pre-round
    # operands to bf16-hi+lo representable values so the fp32r matmul
    # decomposition is exact.
    Sc = Sm - Sm[:, :1]

    def r16(x):
        h = x.astype(ml_dtypes.bfloat16).astype(np.float64)
        l = (x - h).astype(ml_dtypes.bfloat16).astype(np.float64)
        return h + l

    Sc = r16(Sc)
    logpr = r16(logpm)

    stackL = np.zeros((3 * GRP, NG * Q), np.float32)
    stackR = np.zeros((3 * GRP, NG * GRP * Q), np.float32)
    for g in range(NG):
        for k in range(GRP):
            c = g * GRP + k
            lcol = slice(g * Q, (g + 1) * Q)
            stackL[3 * k + 0, lcol] = 1.0
            stackL[3 * k + 1, lcol] = -Sc[c]
            stackL[3 * k + 2, lcol] = logpr[c]
            rcol = slice(g * GRP * Q + k * Q, g * GRP * Q + (k + 1) * Q)
            stackR[3 * k + 0, rcol] = Sc[c]
            stackR[3 * k + 1, rcol] = 1.0
            stackR[3 * k + 2, rcol] = 1.0

    # U exponent column stack: Send_c - S_j + logp_j  -> [Q, C]
    uexp = (Send[:, None] - Sm + logpm).T.astype(np.float32)

    m_i = np.arange(C)[:, None]
    c_i = np.arange(C)[None, :]
    d2exp = np.where(m_i < c_i, Sendprev[None, :] - Send[:, None], NEG)
    d2exp = d2exp.astype(np.float32)

    # kappa_c = exp(S_{c,0} - Send_{c-1} - logp_r[c,0]): scaling such that
    # W^T row 0 (= exp(S'_i + logp_r[c,0])) times kappa*carry reproduces the
    # rank-1 carry term a_i*carry. Uses the device-rounded logp so the
    # coefficient reconstruction cancels exactly.
    kap = np.exp(np.minimum(Sm[:, 0] - Sendprev - logpr[:, 0], 69.0))
    kappa = kap.reshape(C, 1).astype(np.float32)

    aux12 = np.concatenate([stackL, stackR], axis=1)
    aux128 = np.concatenate([_get_maskb(), uexp], axis=1)
    aux32 = np.concatenate([d2exp, kappa], axis=1)
    return aux12, aux128, aux32


_MASKB = None


def _get_maskb():
    global _MASKB
    if _MASKB is None:
        j = np.arange(Q)[:, None]
        i = np.arange(Q)[None, :]
        one = np.where(i >= j, 0.0, NEG).astype(np.float32)
        _MASKB = np.tile(one, (1, GRP))
    return _MASKB


def _make_in_maps(z, pt):
    preps = [_host_prep(pt[b]) for b in range(B)]
    in_maps = []
    for core in range(N_CORES):
        b, dblk = divmod(core, ND)
        aux12, aux128, aux32 = preps[b]
        z_slab = (
            z[b, :, dblk * DBLK : (dblk + 1) * DBLK]
            .reshape(C, Q, DBLK)
            .transpose(1, 0, 2)
            .reshape(Q, C * DBLK)
            .astype(ml_dtypes.bfloat16)
        )
        in_maps.append({
            "z_s": np.ascontiguousarray(z_slab),
            "aux12": aux12,
            "aux128": aux128,
            "aux32": aux32,
        })
    return in_maps


def _unpack_out(res_core):
    """out_s [Q, C*DBLK] position-major -> [L, DBLK]."""
    return (
        res_core.reshape(Q, C, DBLK).transpose(1, 0, 2).reshape(L, DBLK)
    )


def _install_ntff_shim():
    """Enable NTFF profiling under axon: shim the missing antenv.axon_hooks
    module and register the ctypes hook from trn_boot; skip the fileshare
    artifact upload (no bucket in this container)."""
    import sys
    import types
    import antenv

    if "antenv.axon_hooks" not in sys.modules:
        mod = types.ModuleType("antenv.axon_hooks")
        hook_box = [None]
        mod.set_axon_ntff_profile_hook = lambda h: hook_box.__setitem__(0, h)
        mod.get_axon_ntff_profile_hook = lambda: hook_box[0]
        mod._hook_box = hook_box
        sys.modules["antenv.axon_hooks"] = mod
        antenv.axon_hooks = mod
    mod = sys.modules["antenv.axon_hooks"]
    if mod.get_axon_ntff_profile_hook() is None:
        from trn_agent_boot.trn_boot import _ntff_profile_via_ctypes

        mod.set_axon_ntff_profile_hook(
            _ntff_profile_via_ctypes("/opt/axon/libaxon_pjrt.so")
        )
    import concourse.bass_utils as bu

    bu.upload_artifacts = lambda tmpdir: f"local://{tmpdir}"


def kernel(z, pt):
    global LAST_EXEC_NS
    from concourse.bass_utils import run_bass_kernel_spmd

    z = np.asarray(z, dtype=np.float32)
    pt = np.asarray(pt, dtype=np.float32)

    if "nc" not in _CTX:
        _CTX["nc"] = _build_program()
    nc = _CTX["nc"]

    in_maps = _make_in_maps(z, pt)

    trace = bool(int(os.environ.get("BASS_KERNEL_TRACE", "0")))
    if trace:
        try:
            _install_ntff_shim()
        except Exception:
            trace = False
    tmpdir = os.environ.get("BASS_KERNEL_TRACE_DIR") or None
    res = run_bass_kernel_spmd(
        nc, in_maps, list(range(N_CORES)), trace=trace, tmpdir=tmpdir
    )
    LAST_EXEC_NS = res.exec_time_ns

    out = np.empty((B, L, D), np.float32)
    for core in range(N_CORES):
        b, dblk = divmod(core, ND)
        out[b, :, dblk * DBLK : (dblk + 1) * DBLK] = _unpack_out(
            res.results[core]["out_s"]
        )
    return out


# revision 31
# speedup vs baseline: 2.8306x; 1.0673x over previous
"""Trainium2 Bass kernel for DeChunking EMA (lower-triangular decay matmul).

Math: out[b,i,:] = sum_{j<=i} exp(S_i - S_j) * p_j * z[b,j,:],
with S = cumsum(log(clip(1-p))). Computed chunked-scan style (Mamba-SSD):

  - L split into C=32 chunks of Q=128.
  - Intra-chunk: out_intra = W_c^T.T @ z_c with
      W_c^T[j,i] = exp(S'_i - S'_j + log p_j) (masked to i>=j),
    where S' is S re-centered per chunk (only within-chunk differences
    matter, and small magnitudes survive the PE's fp32r mantissa split).
    The delta matrix is produced on PE by a block-diagonal stacked fp32r
    matmul: delta = 1*S'_i + (-S'_j)*1 + logp_j*1, K=3 per chunk -> K=12
    block-diagonal over a group of 4 chunks ([128,512] PSUM, 1 cycle/row).
  - Inter-chunk: chunk states H_c = U_c^T @ z_c accumulate into one
    [32,192] PSUM tile via a block-diagonal U (zero except column c of
    each [128,32] slab), then one [32,32] decay matmul forms all carry-in
    rows (carry = M2^T @ H), applied per chunk as a rank-1 PSUM-accumulated
    matmul out += A_c (x) carry_c.

All exp inputs are <= 0 by construction, so nothing overflows. The decay
weights / z / state operands run in bf16 on the PE (fp32 PSUM accumulate);
the delta stack runs in fp32r (S' re-centered + pre-rounded to bf16 hi+lo).

DRAM layouts are position-major ([Q, C*DBLK]) so every DMA moves >=3 KiB
contiguous per partition.

Sharding (8 cores, no collectives): core = (batch b in {0,1}) x (one of 4
D-blocks of 192). Each core reads z[b, :, blk] and pt[b] only.
"""

import os
import numpy as np
import ml_dtypes

B, L, D = 2, 4096, 768
Q = 128
C = L // Q           # 32 chunks
ND = 4               # D blocks per batch
DBLK = D // ND       # 192
GRP = 4              # chunks per delta/exp group
NG = C // GRP        # 8 groups
NEG = -3.0e38
N_CORES = 8
NZDMA = 4            # z-load / out-store DMA splits

_CTX = {}
LAST_EXEC_NS = None


def _build_program():
    import concourse.bacc as bacc
    import concourse.mybir as mybir
    from concourse import tile

    f32 = mybir.dt.float32
    f32r = mybir.dt.float32r
    bf16 = mybir.dt.bfloat16
    nc = bacc.Bacc("TRN2", target_bir_lowering=False, debug=False,
                   num_devices=N_CORES)

    FD = C * DBLK  # 6144 free elems in the big position-major tiles
    SLAB = C // NZDMA          # 8 chunks per z slab tile
    ZSL = SLAB * DBLK          # free elems per slab
    z_s = nc.dram_tensor("z_s", [Q, FD], bf16, kind="ExternalInput")
    # packed aux inputs: one DMA per partition-count class
    aux12 = nc.dram_tensor("aux12", [3 * GRP, NG * Q + NG * GRP * Q], f32r,
                           kind="ExternalInput")
    aux128 = nc.dram_tensor("aux128", [Q, GRP * Q + C], f32,
                            kind="ExternalInput")
    aux32 = nc.dram_tensor("aux32", [C, C], f32, kind="ExternalInput")
    out_s = nc.dram_tensor("out_s", [Q, FD], f32, kind="ExternalOutput")

    Exp = mybir.ActivationFunctionType.Exp

    with tile.TileContext(nc) as tc:
        with (
            tc.tile_pool(name="zp", bufs=NZDMA) as zp,
            tc.tile_pool(name="wp", bufs=NG) as wp,
            tc.tile_pool(name="sp", bufs=1) as sp,
            tc.tile_pool(name="dps", bufs=3, space="PSUM") as dps,
            tc.tile_pool(name="ops", bufs=3, space="PSUM") as ops,
            tc.tile_pool(name="hps", bufs=1, space="PSUM") as hps,
        ):
            # z slabs stream first on sync (they gate the H state matmuls);
            # aux12 follows (the delta matmuls need it only once H is done)
            zsl = []
            for s in range(NZDMA):
                t = zp.tile([Q, ZSL], bf16, tag="z")
                nc.sync.dma_start(t[:], z_s[:, s * ZSL : (s + 1) * ZSL])
                zsl.append(t)
            a12 = sp.tile([3 * GRP, NG * Q + NG * GRP * Q], f32r, tag="a12")
            nc.sync.dma_start(a12[:], aux12[:])

            def zchunk(c):
                s, r = divmod(c, SLAB)
                return zsl[s][:, r * DBLK : (r + 1) * DBLK]

            # remaining aux loads ride the Activation HWDGE queue in parallel
            a128 = sp.tile([Q, GRP * Q + C], f32, tag="a128")
            nc.scalar.dma_start(a128[:], aux128[:])
            a32 = sp.tile([C, C], f32, tag="a32")
            nc.scalar.dma_start(a32[:], aux32[:])
            sL = a12[:, 0 : NG * Q]
            sR = a12[:, NG * Q :]
            mb = a128[:, 0 : GRP * Q]
            ue = a128[:, GRP * Q :]
            d2 = a32[:, 0:C]

            # U block-diagonal [Q, C*C] bf16: zero it, exp the [Q, C] column
            # stack, scatter onto the diagonal (stride C+1 in the free dim)
            Ublk = sp.tile([Q, C * C], bf16, tag="Ublk")
            nc.gpsimd.memset(Ublk[:], 0.0)
            Us = sp.tile([Q, C], bf16, tag="Us")
            nc.scalar.activation(Us[:], ue, Exp)
            nc.scalar.copy(Ublk[:, 0 : C * C : C + 1], Us[:])
            M2 = sp.tile([C, C], bf16, tag="M2")
            nc.scalar.activation(M2[:], d2, Exp)

            # PE clock warmup: back-to-back dense matmuls on junk data during
            # the input-DMA window flip the HAM gate to 2.4 GHz before the
            # real work arrives (the real matmuls alone are too sparse in
            # array-duty to flip it); the real H matmuls then sustain it
            wm_sb = sp.tile([Q, 2 * DBLK], bf16, tag="wm_sb")
            nc.gpsimd.memset(wm_sb[:], 1.0)
            wm_ps = ops.tile([Q, 2 * DBLK], f32, tag="o")
            for _ in range(14):
                nc.tensor.matmul(wm_ps[:], wm_sb[:, 0:Q], wm_sb[:])

            # H state matmuls chase the z slabs as they land
            h_ps = hps.tile([C, DBLK], f32, tag="h")
            for c in range(C):
                nc.tensor.matmul(
                    h_ps[:],
                    Ublk[:, c * C : (c + 1) * C],
                    zchunk(c),
                    start=(c == 0), stop=(c == C - 1),
                    skip_group_check=True,
                )

            H = sp.tile([C, DBLK], bf16, tag="H")
            nc.vector.tensor_copy(H[:], h_ps[:])
            c_ps = hps.tile([C, DBLK], f32, tag="cps")
            nc.tensor.matmul(c_ps[:], M2[:], H[:])
            # kappa is pre-folded into M2 host-side, so c_ps already holds
            # kappa*carry; cast it and fold into row 0 of each z slab
            # (out += a (x) carry == W^T row 0 applying the rank-1 update
            # once z[0] += kappa*carry)
            cfk = sp.tile([C, DBLK], bf16, tag="cfk")
            nc.vector.tensor_copy(cfk[:], c_ps[:])
            for s in range(NZDMA):
                nc.gpsimd.dma_start(
                    zsl[s][0:1, :],
                    cfk[s * SLAB : (s + 1) * SLAB, :],
                    accum_op=mybir.AluOpType.add,
                )

            # W^T delta matmuls (block-diagonal K=12 fp32r, N=512 -> 1
            # cycle/row) + mask + exp fill the PE while the fold completes;
            # intra-chunk output matmuls trail two delta groups behind
            wT = []
            osb = sp.tile([Q, FD], f32, tag="osb")

            def delta_group(g):
                dp = dps.tile([Q, GRP * Q], f32, tag="dp")
                nc.tensor.matmul(
                    dp[:],
                    sL[:, g * Q : (g + 1) * Q],
                    sR[:, g * GRP * Q : (g + 1) * GRP * Q],
                )
                nc.vector.tensor_add(dp[:], dp[:], mb)
                w4 = wp.tile([Q, GRP * Q], bf16, tag="w4")
                nc.scalar.activation(w4[:], dp[:], Exp)
                wT.append(w4)

            def out_pair(p):
                o_ps = ops.tile([Q, 2 * DBLK], f32, tag="o")
                for h in range(2):
                    c = 2 * p + h
                    g, k = divmod(c, GRP)
                    nc.tensor.matmul(
                        o_ps[:, h * DBLK : (h + 1) * DBLK],
                        wT[g][:, k * Q : (k + 1) * Q],
                        zchunk(c),
                    )
                osl = slice(2 * p * DBLK, (2 * p + 2) * DBLK)
                if p % 2 == 0:
                    nc.scalar.copy(osb[:, osl], o_ps[:])
                else:
                    nc.vector.tensor_copy(osb[:, osl], o_ps[:])

            delta_group(0)
            delta_group(1)
            for g in range(2, NG):
                out_pair(2 * (g - 2))
                out_pair(2 * (g - 2) + 1)
                delta_group(g)
            for p in range(2 * (NG - 2), C // 2):
                out_pair(p)

            ssl = FD // NZDMA
            for s in range(NZDMA):
                nc.sync.dma_start(
                    out_s[:, s * ssl : (s + 1) * ssl],
                    osb[:, s * ssl : (s + 1) * ssl],
                )

    nc.compile()
    return nc


def _host_prep(pt_b):
    """Per-batch host-side prep of the small scan operands. pt_b: [L] f32."""
    pt_b = pt_b.astype(np.float64)
    decay = np.clip(1.0 - pt_b, 1e-12, None)
    S = np.cumsum(np.log(decay))
    logp = np.log(np.maximum(pt_b, 1e-38))
    Send = S[Q - 1 :: Q]
    Sendprev = np.concatenate([[0.0], Send[:-1]])

    Sm = S.reshape(C, Q)
    logpm = logp.reshape(C, Q)
    # Re-center S within each chunk (see module docstring) and pre-round
    # operands to bf16-hi+lo representable values so the fp32r matmul
    # decomposition is exact.
    Sc = Sm - Sm[:, :1]

    def r16(x):
        h = x.astype(ml_dtypes.bfloat16).astype(np.float64)
        l = (x - h).astype(ml_dtypes.bfloat16).astype(np.float64)
        return h + l

    Sc = r16(Sc)
    logpr = r16(logpm)

    stackL = np.zeros((3 * GRP, NG * Q), np.float32)
    stackR = np.zeros((3 * GRP, NG * GRP * Q), np.float32)
    for g in range(NG):
        for k in range(GRP):
            c = g * GRP + k
            lcol = slice(g * Q, (g + 1) * Q)
            stackL[3 * k + 0, lcol] = 1.0
            stackL[3 * k + 1, lcol] = -Sc[c]
            stackL[3 * k + 2, lcol] = logpr[c]
            rcol = slice(g * GRP * Q + k * Q, g * GRP * Q + (k + 1) * Q)
            stackR[3 * k + 0, rcol] = Sc[c]
            stackR[3 * k + 1, rcol] = 1.0
            stackR[3 * k + 2, rcol] = 1.0

    # U exponent column stack: Send_c - S_j + logp_j  -> [Q, C]
    uexp = (Send[:, None] - Sm + logpm).T.astype(np.float32)

    m_i = np.arange(C)[:, None]
    c_i = np.arange(C)[None, :]
    d2exp = np.where(m_i < c_i, Sendprev[None, :] - Send[:, None], NEG)
    d2exp = d2exp.astype(np.float32)

    # log kappa_c = S_{c,0} - Send_{c-1} - logp_r[c,0]: scaling such that
    # W^T row 0 (= exp(S'_i + logp_r[c,0])) times kappa*carry reproduces the
    # rank-1 carry term a_i*carry. Uses the device-rounded logp so the
    # coefficient reconstruction cancels exactly. Folded into the M2 decay
    # matrix exponents host-side (column c of d2exp).
    logkap = np.minimum(Sm[:, 0] - Sendprev - logpr[:, 0], 69.0)
    d2exp = (d2exp + logkap[None, :]).astype(np.float32)

    aux12 = np.concatenate([stackL, stackR], axis=1)
    aux128 = np.concatenate([_get_maskb(), uexp], axis=1)
    aux32 = d2exp
    return aux12, aux128, aux32


_MASKB = None


def _get_maskb():
    global _MASKB
    if _MASKB is None:
        j = np.arange(Q)[:, None]
        i = np.arange(Q)[None, :]
        one = np.where(i >= j, 0.0, NEG).astype(np.float32)
        _MASKB = np.tile(one, (1, GRP))
    return _MASKB


def _make_in_maps(z, pt):
    preps = [_host_prep(pt[b]) for b in range(B)]
    in_maps = []
    for core in range(N_CORES):
        b, dblk = divmod(core, ND)
        aux12, aux128, aux32 = preps[b]
        z_slab = (
            z[b, :, dblk * DBLK : (dblk + 1) * DBLK]
            .reshape(C, Q, DBLK)
            .transpose(1, 0, 2)
            .reshape(Q, C * DBLK)
            .astype(ml_dtypes.bfloat16)
        )
        in_maps.append({
            "z_s": np.ascontiguousarray(z_slab),
            "aux12": aux12,
            "aux128": aux128,
            "aux32": aux32,
        })
    return in_maps


def _unpack_out(res_core):
    """out_s [Q, C*DBLK] position-major -> [L, DBLK]."""
    return (
        res_core.reshape(Q, C, DBLK).transpose(1, 0, 2).reshape(L, DBLK)
    )


def _install_ntff_shim():
    """Enable NTFF profiling under axon: shim the missing antenv.axon_hooks
    module and register the ctypes hook from trn_boot; skip the fileshare
    artifact upload (no bucket in this container)."""
    import sys
    import types
    import antenv

    if "antenv.axon_hooks" not in sys.modules:
        mod = types.ModuleType("antenv.axon_hooks")
        hook_box = [None]
        mod.set_axon_ntff_profile_hook = lambda h: hook_box.__setitem__(0, h)
        mod.get_axon_ntff_profile_hook = lambda: hook_box[0]
        mod._hook_box = hook_box
        sys.modules["antenv.axon_hooks"] = mod
        antenv.axon_hooks = mod
    mod = sys.modules["antenv.axon_hooks"]
    if mod.get_axon_ntff_profile_hook() is None:
        from trn_agent_boot.trn_boot import _ntff_profile_via_ctypes

        mod.set_axon_ntff_profile_hook(
            _ntff_profile_via_ctypes("/opt/axon/libaxon_pjrt.so")
        )
    import concourse.bass_utils as bu

    bu.upload_artifacts = lambda tmpdir: f"local://{tmpdir}"


def kernel(z, pt):
    global LAST_EXEC_NS
    from concourse.bass_utils import run_bass_kernel_spmd

    z = np.asarray(z, dtype=np.float32)
    pt = np.asarray(pt, dtype=np.float32)

    if "nc" not in _CTX:
        _CTX["nc"] = _build_program()
    nc = _CTX["nc"]

    in_maps = _make_in_maps(z, pt)

    trace = bool(int(os.environ.get("BASS_KERNEL_TRACE", "0")))
    if trace:
        try:
            _install_ntff_shim()
        except Exception:
            trace = False
    tmpdir = os.environ.get("BASS_KERNEL_TRACE_DIR") or None
    res = run_bass_kernel_spmd(
        nc, in_maps, list(range(N_CORES)), trace=trace, tmpdir=tmpdir
    )
    LAST_EXEC_NS = res.exec_time_ns

    out = np.empty((B, L, D), np.float32)
    for core in range(N_CORES):
        b, dblk = divmod(core, ND)
        out[b, :, dblk * DBLK : (dblk + 1) * DBLK] = _unpack_out(
            res.results[core]["out_s"]
        )
    return out
